# revision 1
# baseline (speedup 1.0000x reference)
"""DetectionBEVLoss Trainium2 kernel: 8-core data-parallel (1 batch/core).

Layout: per core 65536 elements as [128 partitions, 512 free]. Host packs all
inputs into one fp16 array [128, 32, 512] per core (slot map below). Rotated
IoU uses a branch-free Liang-Barsky edge-clip formulation (each quad's edges
clipped against the other box in that box's axis-aligned frame; boundary line
integral x dy - y dx is rotation invariant, evaluated in the target frame).
"""
import math

import ml_dtypes
import numpy as np

import concourse.bacc as bacc
import concourse.bass as bass
import concourse.mybir as mybir
import concourse.tile as tile
from concourse.bass_utils import run_bass_kernel_spmd

F16 = mybir.dt.float16
F32 = mybir.dt.float32
OP = mybir.AluOpType
AF = mybir.ActivationFunctionType

P = 128          # partitions
FW = 512         # free width per partition (128*512 = 65536 elems/core)
NCH = 2          # free-dim chunks
FC = FW // NCH   # chunk width

# slot map in the packed fp16 input [128, 32, 512]
# 0-8: reg_pred c0..c8 | 9-17: reg_targets c0..c8 | 18: iou_pred | 19: iou_targets
# 20: cls_targets (as f16) | 21: reg_weights (as f16) | 22-31: cls_pred c0..c9
NSLOT = 32

EPS = 1e-7


def _ap(t, s0, slot_dims, col0, ncol, colstep=1):
    """Manual AP into tile t ([128, S, W]): base slot s0, then
    (slot_step, count) dims, innermost column dim. Slot stride taken
    from the tile's own AP (W elements)."""
    ss = t.ap[-2][0]
    ap = [list(t.ap[0])] + [[s * ss, c] for s, c in slot_dims] + [[colstep, ncol]]
    return bass.AP(tensor=t.tensor, offset=t.offset + s0 * ss + col0, ap=ap)


def build_bass():
    nc = bacc.Bacc("TRN2", target_bir_lowering=False, debug=False)
    h16 = nc.declare_dram_parameter("h16", [P, NSLOT, FW], F16, isOutput=False)
    outp = nc.declare_dram_parameter("out", [1, 32], F32, isOutput=True)

    with tile.TileContext(nc) as tc:
        with (
            tc.tile_pool(name="main", bufs=1) as pool,
            tc.tile_pool(name="small", bufs=1) as spool,
            tc.tile_pool(name="ps", bufs=1, space="PSUM") as ppool,
        ):
            IN = pool.tile([P, NSLOT, FW], F16)
            # DMA in: geometry slots first, cls last
            nc.sync.dma_start(out=IN[:, 0:22, :], in_=h16[:, 0:22, :])
            nc.sync.dma_start(out=IN[:, 22:32, :], in_=h16[:, 22:32, :])

            pibias = spool.tile([P, 1], F32)
            nc.vector.memset(pibias, math.pi / 2)
            ones = spool.tile([P, 1], F32)
            nc.vector.memset(ones, 1.0)
            ACC = spool.tile([P, 32], F32)
            nc.vector.memset(ACC, 0.0)

            # ---- full-width trig / halves / cd-sd / dxy ----
            # sin/cos via Taylor poly on DVE (yaw in [0,1); ACT's sin table
            # can't share a table-set with exp/ln)
            TR = pool.tile([P, 4, FW], F16)   # cosp sinp cost sint
            X2 = pool.tile([P, 2, FW], F16)   # yaw^2 for p and t
            YAWS = _ap(IN, 6, [(9, 2)], 0, FW)  # slots 6, 15
            nc.vector.tensor_tensor(out=X2, in0=YAWS, in1=YAWS, op=OP.mult)
            SPH = pool.tile([P, 2, FW], F16)
            nc.vector.tensor_scalar(out=SPH, in0=X2, scalar1=1.0 / 120,
                                    scalar2=-1.0 / 6, op0=OP.mult, op1=OP.add)
            nc.vector.tensor_tensor(out=SPH, in0=SPH, in1=X2, op=OP.mult)
            nc.vector.scalar_tensor_tensor(out=_ap(TR, 1, [(2, 2)], 0, FW), in0=SPH,
                                           scalar=1.0, in1=YAWS, op0=OP.add, op1=OP.mult)
            CPH = pool.tile([P, 2, FW], F16)
            nc.vector.tensor_scalar(out=CPH, in0=X2, scalar1=-1.0 / 720,
                                    scalar2=1.0 / 24, op0=OP.mult, op1=OP.add)
            nc.vector.tensor_tensor(out=CPH, in0=CPH, in1=X2, op=OP.mult)
            nc.vector.tensor_scalar(out=CPH, in0=CPH, scalar1=-0.5,
                                    scalar2=None, op0=OP.add)
            nc.vector.tensor_tensor(out=CPH, in0=CPH, in1=X2, op=OP.mult)
            nc.vector.tensor_scalar(out=_ap(TR, 0, [(2, 2)], 0, FW), in0=CPH,
                                    scalar1=1.0, scalar2=None, op0=OP.add)

            HV = pool.tile([P, 4, FW], F16)   # lht wht lhp whp
            # IN slots 12,13 = [wht,lht]*2 -> write reversed into HV slots 1,0
            nc.vector.tensor_scalar(
                out=_ap(HV, 1, [(-1, 2)], 0, FW), in0=IN[:, 12:14, :],
                scalar1=0.5, scalar2=None, op0=OP.mult)
            nc.vector.tensor_scalar(
                out=_ap(HV, 3, [(-1, 2)], 0, FW), in0=IN[:, 3:5, :],
                scalar1=0.5, scalar2=None, op0=OP.mult)

            CS = pool.tile([P, 2, FW], F16)   # cd sd
            TP = pool.tile([P, 2, FW], F16)
            TQ = pool.tile([P, 2, FW], F16)
            # TP = [cp*ct, sp*st]
            nc.vector.tensor_tensor(out=TP, in0=TR[:, 0:2, :], in1=TR[:, 2:4, :], op=OP.mult)
            # TQ = [sp*ct, cp*st]  (in0 = TR slots [1,0])
            nc.vector.tensor_tensor(out=TQ, in0=_ap(TR, 1, [(-1, 2)], 0, FW),
                                    in1=TR[:, 2:4, :], op=OP.mult)
            nc.vector.tensor_tensor(out=CS[:, 0, :], in0=TP[:, 0, :], in1=TP[:, 1, :], op=OP.add)
            nc.vector.tensor_tensor(out=CS[:, 1, :], in0=TQ[:, 0, :], in1=TQ[:, 1, :], op=OP.subtract)

            DXY = pool.tile([P, 2, FW], F16)  # dx dy
            nc.vector.tensor_tensor(out=DXY, in0=IN[:, 0:2, :], in1=IN[:, 9:11, :], op=OP.subtract)

            ACS = pool.tile([P, 4, FW], F16)  # |cp| |sp| |ct| |st|
            nc.scalar.activation(ACS, TR, AF.Abs)

            for j in range(NCH):
                c0 = j * FC
                cols = slice(c0, c0 + FC)

                def inp(s):
                    return IN[:, s, cols]

                def hv(s):
                    return HV[:, s, cols]

                # ---------- corner transforms ----------
                DC = pool.tile([P, 4, FC], F16, tag="DC")  # dcxA dcyA dcxB dcyB
                # PTall = [ct*dx, ct*dy, cp*dx, cp*dy]; QTall = [st*..., sp*...]
                PTall = pool.tile([P, 4, FC], F16, tag="PT")
                QTall = pool.tile([P, 4, FC], F16, tag="QT")
                nc.vector.tensor_tensor(out=PTall,
                                        in0=_ap(DXY, 0, [(0, 2), (1, 2)], c0, FC),
                                        in1=_ap(TR, 2, [(-2, 2), (0, 2)], c0, FC), op=OP.mult)
                nc.vector.tensor_tensor(out=QTall,
                                        in0=_ap(DXY, 0, [(0, 2), (1, 2)], c0, FC),
                                        in1=_ap(TR, 3, [(-2, 2), (0, 2)], c0, FC), op=OP.mult)
                # dcx = c*dx + s*dy ; dcy = c*dy - s*dx  (both directions at once)
                nc.vector.tensor_tensor(out=_ap(DC, 0, [(2, 2)], 0, FC),
                                        in0=_ap(PTall, 0, [(2, 2)], 0, FC),
                                        in1=_ap(QTall, 1, [(2, 2)], 0, FC), op=OP.add)
                nc.vector.tensor_tensor(out=_ap(DC, 1, [(2, 2)], 0, FC),
                                        in0=_ap(PTall, 1, [(2, 2)], 0, FC),
                                        in1=_ap(QTall, 0, [(2, 2)], 0, FC), op=OP.subtract)

                # UVX: cd*[lhp,whp,lht,wht], sd*[whp,lhp,wht,lht]
                UVX = pool.tile([P, 8, FC], F16, tag="UV")
                nc.vector.tensor_tensor(out=UVX[:, 0:4, :],
                                        in0=_ap(CS, 0, [(0, 4)], c0, FC),
                                        in1=_ap(HV, 2, [(-2, 2), (1, 2)], c0, FC), op=OP.mult)
                nc.vector.tensor_tensor(out=UVX[:, 4:8, :],
                                        in0=_ap(CS, 1, [(0, 4)], c0, FC),
                                        in1=_ap(HV, 3, [(-1, 4)], c0, FC), op=OP.mult)
                # SC layout: [sA, sB, sD, sC, pB, pA, pC, pD]
                SC = pool.tile([P, 8, FC], F16, tag="SC")
                nc.vector.tensor_tensor(out=_ap(SC, 0, [(2, 4)], 0, FC),
                                        in0=_ap(UVX, 0, [(2, 2), (5, 2)], 0, FC),
                                        in1=_ap(UVX, 4, [(2, 2), (-3, 2)], 0, FC), op=OP.add)
                nc.vector.tensor_tensor(out=_ap(SC, 1, [(2, 4)], 0, FC),
                                        in0=_ap(UVX, 0, [(2, 2), (5, 2)], 0, FC),
                                        in1=_ap(UVX, 4, [(2, 2), (-3, 2)], 0, FC), op=OP.subtract)

                # corners: slots 0-3 AX, 4-7 AY, 8-11 BX, 12-15 BY  (CW order)
                # AX = dcx + [sA,-sB,-sA,sB] ; AY = dcy + [sC,-sD,-sC,sD]
                # BX = dcx2 + [-pA,pB,pA,-pB]; BY = dcy2 + [pC,-pD,-pC,pD]
                CRN = pool.tile([P, 16, FC], F16, tag="CRN")
                bcast = lambda src, n: _ap(src[0], src[1], [(0, n)], c0, FC)

                def corner2(dst0, step, dcslot, scslot, scstep, op):
                    # CRN[{dst0, dst0+step}] = DC[dcslot] op SC[{scslot, scslot+scstep}]
                    nc.vector.tensor_tensor(
                        out=_ap(CRN, dst0, [(step, 2)], 0, FC),
                        in0=_ap(DC, dcslot, [(0, 2)], 0, FC),
                        in1=_ap(SC, scslot, [(scstep, 2)], 0, FC), op=op)

                corner2(0, 3, 0, 0, 1, OP.add)        # AX0=dcx+sA, AX3=dcx+sB
                corner2(1, 1, 0, 1, -1, OP.subtract)  # AX1=dcx-sB, AX2=dcx-sA
                corner2(4, 3, 1, 3, -1, OP.add)       # AY0=dcy+sC, AY3=dcy+sD
                corner2(5, 1, 1, 2, 1, OP.subtract)   # AY1=dcy-sD, AY2=dcy-sC
                corner2(9, 1, 2, 4, 1, OP.add)        # BX1=dcx2+pB, BX2=dcx2+pA
                corner2(8, 3, 2, 5, -1, OP.subtract)  # BX0=dcx2-pA, BX3=dcx2-pB
                corner2(12, 3, 3, 6, 1, OP.add)       # BY0=dcy2+pC, BY3=dcy2+pD
                corner2(13, 1, 3, 7, -1, OP.subtract) # BY1=dcy2-pD, BY2=dcy2-pC

                # ---------- edge vectors, reciprocals (per 4-slot group) ----------
                # boxes are parallelograms: edge 2 = -edge 0, edge 3 = -edge 1,
                # so only edges 0,1 need the reciprocal; 2,3 are negated copies
                RD = pool.tile([P, 16, FC], F16, tag="RD")
                for g in range(4):
                    b = g * 4
                    D32g = pool.tile([P, 2, FC], F32, tag="D32g")
                    nc.vector.tensor_tensor(out=D32g, in0=CRN[:, b + 1:b + 3, :],
                                            in1=CRN[:, b:b + 2, :], op=OP.subtract)
                    # keep D away from exact 0: fp16 corners cancel exactly for
                    # near-parallel edges; approx reciprocal of 0 is NaN
                    nc.vector.tensor_scalar(out=D32g, in0=D32g, scalar1=1e-12,
                                            scalar2=None, op0=OP.add)
                    R32g = pool.tile([P, 2, FC], F32, tag="R32g")
                    nc.vector.reciprocal_approx_fast(out=R32g.rearrange("p a b -> p (a b)"),
                                                     in_=D32g.rearrange("p a b -> p (a b)"))
                    nc.vector.tensor_scalar(out=RD[:, b:b + 2, :], in0=R32g,
                                            scalar1=-8000.0, scalar2=8000.0,
                                            op0=OP.max, op1=OP.min)
                    nc.vector.tensor_scalar(out=RD[:, b + 2:b + 4, :], in0=RD[:, b:b + 2, :],
                                            scalar1=-1.0, scalar2=None, op0=OP.mult)

                # ---------- Liang-Barsky slab clip ----------
                # slot groups: 0-3 use L=lht(HV0), 4-7 wht(HV1), 8-11 lhp(HV2), 12-15 whp(HV3)
                # lo = -(L|r| + C r), hi = L|r| - C r  (r clamped finite -> no NaN)
                # |r| and L*|r| identical for opposite edges: compute on 8 slots,
                # read back through a repeat-AP
                RA = pool.tile([P, 4, 2, FC], F16, tag="RA8")
                nc.scalar.activation(RA, _ap(RD, 0, [(4, 4), (1, 2)], 0, FC), AF.Abs)
                Q1 = pool.tile([P, 16, FC], F16, tag="NB")
                nc.vector.tensor_tensor(out=Q1, in0=CRN, in1=RD, op=OP.mult)   # C*r
                RL = pool.tile([P, 4, 2, FC], F16, tag="RL8")
                nc.vector.tensor_tensor(out=RL, in0=_ap(HV, 0, [(1, 4), (0, 2)], c0, FC),
                                        in1=RA, op=OP.mult)                    # L*|r|
                RLrep = _ap(RL, 0, [(2, 4), (0, 2), (1, 2)], 0, FC)
                HI = pool.tile([P, 16, FC], F16, tag="NA")
                nc.vector.tensor_tensor(out=_ap(HI, 0, [(4, 4), (2, 2), (1, 2)], 0, FC),
                                        in0=RLrep,
                                        in1=_ap(Q1, 0, [(4, 4), (2, 2), (1, 2)], 0, FC),
                                        op=OP.subtract)
                TQ2 = pool.tile([P, 16, FC], F16, tag="P2")
                nc.vector.tensor_tensor(out=_ap(TQ2, 0, [(4, 4), (2, 2), (1, 2)], 0, FC),
                                        in0=RLrep,
                                        in1=_ap(Q1, 0, [(4, 4), (2, 2), (1, 2)], 0, FC),
                                        op=OP.add)                             # -lo
                # t0 = max(-min(tqx,tqy), 0) ; t1 = min(min(hix,hiy), 1)
                T0 = pool.tile([P, 8, FC], F16, tag="P1")
                T1 = pool.tile([P, 8, FC], F16, tag="NB")
                nc.vector.tensor_tensor(out=T0, in0=_ap(TQ2, 0, [(8, 2), (1, 4)], 0, FC),
                                        in1=_ap(TQ2, 4, [(8, 2), (1, 4)], 0, FC), op=OP.min)
                nc.vector.tensor_scalar(out=T0, in0=T0, scalar1=-1.0, scalar2=0.0,
                                        op0=OP.mult, op1=OP.max)
                nc.vector.tensor_tensor(out=T1, in0=_ap(HI, 0, [(8, 2), (1, 4)], 0, FC),
                                        in1=_ap(HI, 4, [(8, 2), (1, 4)], 0, FC), op=OP.min)
                nc.vector.tensor_scalar(out=T1, in0=T1, scalar1=1.0, scalar2=None, op0=OP.min)
                SEG = pool.tile([P, 8, FC], F16, tag="SEG")
                nc.vector.tensor_tensor(out=SEG, in0=T1, in1=T0, op=OP.subtract)
                nc.vector.tensor_scalar(out=SEG, in0=SEG, scalar1=0.0, scalar2=None, op0=OP.max)

                # ---------- cross products (dir A) + accumulate intersection ----------
                CR1 = pool.tile([P, 4, FC], F16, tag="CR1")
                CR2 = pool.tile([P, 4, FC], F16, tag="CR2")
                nc.vector.tensor_tensor(out=CR1[:, 0:3, :], in0=CRN[:, 0:3, :],
                                        in1=CRN[:, 5:8, :], op=OP.mult)
                nc.vector.tensor_tensor(out=CR1[:, 3, :], in0=CRN[:, 3, :],
                                        in1=CRN[:, 4, :], op=OP.mult)
                nc.vector.tensor_tensor(out=CR2[:, 0:3, :], in0=CRN[:, 4:7, :],
                                        in1=CRN[:, 1:4, :], op=OP.mult)
                nc.vector.tensor_tensor(out=CR2[:, 3, :], in0=CRN[:, 7, :],
                                        in1=CRN[:, 0, :], op=OP.mult)
                nc.vector.tensor_tensor(out=CR1, in0=CR1, in1=CR2, op=OP.subtract)
                CA = pool.tile([P, 4, FC], F16, tag="CA")
                nc.vector.tensor_tensor(out=CA, in0=CR1, in1=SEG[:, 0:4, :], op=OP.mult)
                CAT = pool.tile([P, 2, FC], F16, tag="CAT")
                nc.vector.tensor_tensor(out=CAT, in0=CA[:, 0:2, :], in1=CA[:, 2:4, :], op=OP.add)
                ACA = pool.tile([P, FC], F32, tag="ACA")
                nc.vector.tensor_tensor(out=ACA, in0=CAT[:, 0, :], in1=CAT[:, 1, :], op=OP.add)
                SB2 = pool.tile([P, 2, FC], F16, tag="SB2")
                nc.vector.tensor_tensor(out=SB2, in0=SEG[:, 4:6, :], in1=SEG[:, 6:8, :], op=OP.add)
                SBS = pool.tile([P, FC], F16, tag="SBS")
                nc.vector.tensor_tensor(out=SBS, in0=SB2[:, 0, :], in1=SB2[:, 1, :], op=OP.add)
                M32 = pool.tile([P, FC], F32, tag="M32")
                nc.vector.tensor_tensor(out=M32, in0=hv(0), in1=hv(1), op=OP.mult)  # lht*wht
                MM = pool.tile([P, FC], F32, tag="MM")
                nc.vector.tensor_tensor(out=MM, in0=M32, in1=SBS, op=OP.mult)
                nc.vector.scalar_tensor_tensor(out=ACA, in0=MM, scalar=-2.0, in1=ACA,
                                               op0=OP.mult, op1=OP.add)

                INTER = pool.tile([P, FC], F32, tag="INTER")
                nc.scalar.activation(INTER, ACA, AF.Abs, scale=0.5)
                AP32 = pool.tile([P, FC], F32, tag="AP32")
                nc.vector.tensor_tensor(out=AP32, in0=hv(2), in1=hv(3), op=OP.mult)  # lhp*whp
                U1 = pool.tile([P, FC], F32, tag="U1")
                nc.vector.tensor_tensor(out=U1, in0=AP32, in1=M32, op=OP.add)
                UNION = pool.tile([P, FC], F32, tag="UNION")
                nc.vector.scalar_tensor_tensor(out=UNION, in0=U1, scalar=4.0, in1=INTER,
                                               op0=OP.mult, op1=OP.subtract)
                UC = pool.tile([P, FC], F32, tag="UC")
                nc.vector.tensor_scalar(out=UC, in0=UNION, scalar1=EPS, scalar2=None, op0=OP.max)
                RUC = pool.tile([P, FC], F32, tag="RUC")
                nc.vector.reciprocal_approx_fast(out=RUC, in_=UC)
                IOU = pool.tile([P, FC], F32, tag="IOU")
                nc.vector.tensor_tensor(out=IOU, in0=INTER, in1=RUC, op=OP.mult)
                MU = pool.tile([P, FC], F32, tag="MU")
                nc.vector.tensor_scalar(out=MU, in0=UNION, scalar1=EPS, scalar2=None, op0=OP.is_gt)
                nc.vector.tensor_tensor(out=IOU, in0=IOU, in1=MU, op=OP.mult)

                # ---------- enclosing box diag^2 + center dist (Pool engine) ----------
                PA_ = pool.tile([P, 4, FC], F16, tag="PA_")
                PB_ = pool.tile([P, 4, FC], F16, tag="PB_")
                # PA = [lhp|cp|, whp|sp|, lht|ct|, wht|st|] ; hv order [lht,wht,lhp,whp]
                nc.gpsimd.tensor_tensor(out=PA_, in0=_ap(HV, 2, [(-2, 2), (1, 2)], c0, FC),
                                        in1=ACS[:, :, cols], op=OP.mult)
                nc.gpsimd.tensor_tensor(out=PB_, in0=_ap(HV, 2, [(-2, 2), (1, 2)], c0, FC),
                                        in1=_ap(ACS, 1, [(2, 2), (-1, 2)], c0, FC), op=OP.mult)
                EX = pool.tile([P, 2, FC], F16, tag="EX")  # [ex_p, ex_t]
                EY = pool.tile([P, 2, FC], F16, tag="EY")
                nc.gpsimd.tensor_tensor(out=EX, in0=_ap(PA_, 0, [(2, 2)], 0, FC),
                                        in1=_ap(PA_, 1, [(2, 2)], 0, FC), op=OP.add)
                nc.gpsimd.tensor_tensor(out=EY, in0=_ap(PB_, 0, [(2, 2)], 0, FC),
                                        in1=_ap(PB_, 1, [(2, 2)], 0, FC), op=OP.add)
                PX = _ap(IN, 0, [(9, 2)], c0, FC)   # [xp, xt]
                PY = _ap(IN, 1, [(9, 2)], c0, FC)   # [yp, yt]
                XE = pool.tile([P, 2, FC], F16, tag="XE")
                XD = pool.tile([P, 2, FC], F16, tag="XD")
                YE = pool.tile([P, 2, FC], F16, tag="YE")
                YD = pool.tile([P, 2, FC], F16, tag="YD")
                nc.gpsimd.tensor_tensor(out=XE, in0=PX, in1=EX, op=OP.add)
                nc.gpsimd.tensor_tensor(out=XD, in0=PX, in1=EX, op=OP.subtract)
                nc.gpsimd.tensor_tensor(out=YE, in0=PY, in1=EY, op=OP.add)
                nc.gpsimd.tensor_tensor(out=YD, in0=PY, in1=EY, op=OP.subtract)
                HL = pool.tile([P, 4, FC], F16, tag="HL")  # hx lx hy ly
                nc.vector.tensor_tensor(out=HL[:, 0, :], in0=XE[:, 0, :], in1=XE[:, 1, :], op=OP.max)
                nc.vector.tensor_tensor(out=HL[:, 1, :], in0=XD[:, 0, :], in1=XD[:, 1, :], op=OP.min)
                nc.vector.tensor_tensor(out=HL[:, 2, :], in0=YE[:, 0, :], in1=YE[:, 1, :], op=OP.max)
                nc.vector.tensor_tensor(out=HL[:, 3, :], in0=YD[:, 0, :], in1=YD[:, 1, :], op=OP.min)
                W2 = pool.tile([P, 2, FC], F16, tag="W2")
                nc.gpsimd.tensor_tensor(out=W2, in0=_ap(HL, 0, [(2, 2)], 0, FC),
                                        in1=_ap(HL, 1, [(2, 2)], 0, FC), op=OP.subtract)
                SQ = pool.tile([P, 2, FC], F32, tag="SQ")
                nc.gpsimd.tensor_tensor(out=SQ, in0=W2, in1=W2, op=OP.mult)
                C2 = pool.tile([P, FC], F32, tag="C2")
                nc.gpsimd.tensor_tensor(out=C2, in0=SQ[:, 0, :], in1=SQ[:, 1, :], op=OP.add)
                nc.vector.tensor_scalar(out=C2, in0=C2, scalar1=EPS, scalar2=None, op0=OP.max)
                D2P = pool.tile([P, 2, FC], F32, tag="D2P")
                nc.gpsimd.tensor_tensor(out=D2P, in0=DXY[:, :, cols], in1=DXY[:, :, cols], op=OP.mult)
                D2 = pool.tile([P, FC], F32, tag="D2")
                nc.gpsimd.tensor_tensor(out=D2, in0=D2P[:, 0, :], in1=D2P[:, 1, :], op=OP.add)
                RC2 = pool.tile([P, FC], F32, tag="RC2")
                nc.vector.reciprocal_approx_fast(out=RC2, in_=C2)
                DL = pool.tile([P, FC], F32, tag="DL")
                nc.vector.tensor_tensor(out=DL, in0=D2, in1=RC2, op=OP.mult)
                nc.vector.tensor_tensor(out=DL, in0=DL, in1=IOU, op=OP.subtract)
                wmask = inp(21)
                PR32 = pool.tile([P, FC], F32, tag="PR32")
                nc.vector.tensor_tensor(out=PR32, in0=DL, in1=wmask, op=OP.mult)
                JK32 = pool.tile([P, FC], F32, tag="JK32")
                nc.scalar.activation(JK32, PR32, AF.Copy,
                                     accum_out=ACC[:, 2 + 16 * j:3 + 16 * j])

            # ---- full-width tail: smooth-L1, BCE, focal (independent of geometry) ----
            def inpF(s):
                return IN[:, s, :]

            # ---------- smooth L1 on z,h,vx,vy (Pool) ----------
            DD = pool.tile([P, 4, FW], F16, tag="UV")
            nc.gpsimd.tensor_tensor(out=DD[:, 0, :], in0=inpF(2), in1=inpF(11), op=OP.subtract)
            nc.gpsimd.tensor_tensor(out=DD[:, 1, :], in0=inpF(5), in1=inpF(14), op=OP.subtract)
            nc.gpsimd.tensor_tensor(out=DD[:, 2:4, :], in0=IN[:, 7:9, :],
                                    in1=IN[:, 16:18, :], op=OP.subtract)
            nc.scalar.activation(DD, DD, AF.Abs)
            SLM = pool.tile([P, 4, FW], F16, tag="SEG")
            nc.vector.tensor_scalar(out=SLM, in0=DD, scalar1=1.0, scalar2=None, op0=OP.is_lt)
            AM1 = pool.tile([P, 4, FW], F16, tag="RD")
            nc.vector.tensor_scalar(out=AM1, in0=DD, scalar1=-1.0, scalar2=None, op0=OP.add)
            nc.gpsimd.tensor_tensor(out=AM1, in0=AM1, in1=AM1, op=OP.mult)
            nc.vector.scalar_tensor_tensor(out=AM1, in0=SLM, scalar=0.5, in1=AM1,
                                           op0=OP.mult, op1=OP.mult)
            nc.gpsimd.tensor_tensor(out=DD, in0=DD, in1=AM1, op=OP.add)  # sl1 + 0.5
            PRS = pool.tile([P, 4, FW], F16, tag="CRN")
            nc.vector.tensor_tensor(out=PRS, in0=DD,
                                    in1=_ap(IN, 21, [(0, 4)], 0, FW), op=OP.mult)
            JK16 = pool.tile([P, FW], F16, tag="JK16")
            for k in range(4):
                nc.scalar.activation(JK16, PRS[:, k, :], AF.Copy,
                                     accum_out=ACC[:, 3 + k + 0:4 + k + 0])

            # ---------- BCE on iou head (Pool + ACT) ----------
            BR = pool.tile([P, FW], F16, tag="BR")
            nc.vector.tensor_scalar(out=BR, in0=inpF(18), scalar1=0.0, scalar2=None, op0=OP.max)
            BA = pool.tile([P, FW], F16, tag="BA")
            nc.scalar.activation(BA, inpF(18), AF.Abs)
            BS = pool.tile([P, FW], F16, tag="BS")
            nc.scalar.activation(BS, BA, AF.Exp, scale=-1.0)   # e^{-|x|}
            nc.scalar.activation(BS, BS, AF.Ln, bias=1.0)      # ln(1 + e^{-|x|})
            nc.gpsimd.tensor_tensor(out=BR, in0=BR, in1=BS, op=OP.add)
            BXY = pool.tile([P, FW], F16, tag="BXY")
            nc.gpsimd.tensor_tensor(out=BXY, in0=inpF(18), in1=inpF(19), op=OP.mult)
            nc.gpsimd.tensor_tensor(out=BR, in0=BR, in1=BXY, op=OP.subtract)
            PRB = pool.tile([P, FW], F16, tag="PRB")
            nc.vector.tensor_tensor(out=PRB, in0=BR, in1=inpF(21), op=OP.mult)
            nc.scalar.activation(JK16, PRB, AF.Copy,
                                 accum_out=ACC[:, 7 + 0:8 + 0])

            # ---------- focal ----------
            ET = pool.tile([P, 10, FW], F16, tag="NA")
            nc.scalar.activation(ET, IN[:, 22:32, :], AF.Exp)
            S5 = pool.tile([P, 5, FW], F16, tag="S5")
            nc.vector.tensor_tensor(out=S5, in0=ET[:, 0:5, :], in1=ET[:, 5:10, :], op=OP.add)
            S2 = pool.tile([P, 2, FW], F16, tag="S2")
            nc.vector.tensor_tensor(out=S2, in0=S5[:, 0:2, :], in1=S5[:, 2:4, :], op=OP.add)
            SS = pool.tile([P, FW], F16, tag="SS")
            nc.vector.tensor_tensor(out=SS, in0=S2[:, 0, :], in1=S2[:, 1, :], op=OP.add)
            nc.vector.tensor_tensor(out=SS, in0=SS, in1=S5[:, 4, :], op=OP.add)
            clsf = inpF(20)
            MT = pool.tile([P, 10, FW], F16, tag="NB")
            for c in range(10):
                nc.vector.scalar_tensor_tensor(out=MT[:, c, :], in0=clsf, scalar=float(c),
                                               in1=IN[:, 22 + c, :],
                                               op0=OP.is_equal, op1=OP.mult)
            nc.vector.tensor_tensor(out=S5, in0=MT[:, 0:5, :], in1=MT[:, 5:10, :], op=OP.add)
            nc.vector.tensor_tensor(out=S2, in0=S5[:, 0:2, :], in1=S5[:, 2:4, :], op=OP.add)
            LT = pool.tile([P, FW], F16, tag="LT")
            nc.vector.tensor_tensor(out=LT, in0=S2[:, 0, :], in1=S2[:, 1, :], op=OP.add)
            nc.vector.tensor_tensor(out=LT, in0=LT, in1=S5[:, 4, :], op=OP.add)
            LNS = pool.tile([P, FW], F16, tag="LNS")
            nc.scalar.activation(LNS, SS, AF.Ln)
            LPT = pool.tile([P, FW], F16, tag="LPT")
            nc.vector.tensor_tensor(out=LPT, in0=LT, in1=LNS, op=OP.subtract)
            PTT = pool.tile([P, FW], F16, tag="PTT")
            nc.scalar.activation(PTT, LPT, AF.Exp)
            ONEM = pool.tile([P, FW], F16, tag="ONEM")
            nc.vector.tensor_scalar(out=ONEM, in0=PTT, scalar1=-1.0, scalar2=1.0,
                                    op0=OP.mult, op1=OP.add)
            nc.vector.tensor_tensor(out=ONEM, in0=ONEM, in1=ONEM, op=OP.mult)
            MPOS = pool.tile([P, FW], F16, tag="MPOS")
            nc.vector.tensor_scalar(out=MPOS, in0=clsf, scalar1=0.5, scalar2=None, op0=OP.is_gt)
            nc.vector.tensor_scalar(out=MPOS, in0=MPOS, scalar1=-0.5, scalar2=0.75,
                                    op0=OP.mult, op1=OP.add)
            F1 = pool.tile([P, FW], F16, tag="F1")
            nc.vector.tensor_tensor(out=F1, in0=ONEM, in1=LPT, op=OP.mult)
            nc.vector.tensor_tensor(out=F1, in0=F1, in1=MPOS, op=OP.mult)
            VLD = pool.tile([P, FW], F16, tag="VLD")
            nc.vector.tensor_scalar(out=VLD, in0=clsf, scalar1=-0.5, scalar2=None, op0=OP.is_ge)
            PRF = pool.tile([P, FW], F16, tag="PRF")
            nc.vector.tensor_tensor(out=PRF, in0=F1, in1=VLD, op=OP.mult)
            nc.scalar.activation(JK16, PRF, AF.Copy, scale=-1.0,
                                 accum_out=ACC[:, 0 + 0:1 + 0])
            nc.scalar.activation(JK16, VLD, AF.Copy,
                                 accum_out=ACC[:, 1 + 0:2 + 0])
            nc.scalar.activation(JK16, inpF(21), AF.Copy,
                                 accum_out=ACC[:, 8 + 0:9 + 0])

            # ---------- cross-partition reduce + output ----------
            PS = ppool.tile([1, 32], F32)
            nc.tensor.matmul(PS, ones, ACC, start=True, stop=True)
            OUT = spool.tile([1, 32], F32)
            nc.scalar.copy(out=OUT, in_=PS)
            nc.sync.dma_start(out=outp[:, :], in_=OUT)
    nc.compile()
    return nc


_NC_CACHE = None


def _get_nc():
    global _NC_CACHE
    if _NC_CACHE is None:
        _NC_CACHE = build_bass()
    return _NC_CACHE


def pack_inputs(cls_pred, reg_pred, iou_pred, reg_targets, iou_targets,
                cls_targets, reg_weights):
    """Returns list of 8 per-core input dicts."""
    B = cls_pred.shape[0]
    maps = []
    for b in range(B):
        h = np.empty((NSLOT, P, FW), np.float16)
        h[0:9] = np.asarray(reg_pred[b], np.float32).reshape(9, P, FW)
        h[9:18] = np.asarray(reg_targets[b], np.float32).reshape(9, P, FW)
        h[18] = np.asarray(iou_pred[b], np.float32).reshape(P, FW)
        h[19] = np.asarray(iou_targets[b], np.float32).reshape(P, FW)
        h[20] = np.asarray(cls_targets[b]).astype(np.float32).reshape(P, FW)
        h[21] = np.asarray(reg_weights[b]).astype(np.float32).reshape(P, FW)
        h[22:32] = np.asarray(cls_pred[b], np.float32).reshape(10, P, FW)
        maps.append({"h16": np.ascontiguousarray(h.transpose(1, 0, 2))})
    return maps


def combine(parts):
    """parts: [8, 1, 32] per-core raw sums -> final [7] float32."""
    p = np.asarray(parts, np.float64).sum(0).reshape(2, 16).sum(0)
    focal_s, valid_s, diou_s, z_s, h_s, vx_s, vy_s, bce_s, w_s = p[:9]
    num_pos = max(w_s, 1.0)
    cls_loss = focal_s / max(valid_s, 1.0)
    bev_loss = (diou_s + w_s) / num_pos
    z_loss = (z_s - 0.5 * w_s) / num_pos
    h_loss = (h_s - 0.5 * w_s) / num_pos
    vel_loss = (vx_s + vy_s - w_s) / num_pos
    iou_loss = bce_s / num_pos
    total = cls_loss + 2.0 * bev_loss + z_loss + h_loss + vel_loss + iou_loss
    return np.array([total, cls_loss, bev_loss, z_loss, h_loss, vel_loss, iou_loss],
                    np.float32)


def kernel(cls_pred, reg_pred, iou_pred, reg_targets, iou_targets,
           cls_targets, reg_weights, _trace=False):
    # accept jax or numpy inputs
    cls_pred, reg_pred, iou_pred, reg_targets, iou_targets, cls_targets, reg_weights = (
        np.asarray(a) for a in (cls_pred, reg_pred, iou_pred, reg_targets,
                                iou_targets, cls_targets, reg_weights))
    nc = _get_nc()
    in_maps = pack_inputs(cls_pred, reg_pred, iou_pred, reg_targets,
                          iou_targets, cls_targets, reg_weights)
    res = run_bass_kernel_spmd(nc, in_maps, core_ids=list(range(8)), trace=_trace)
    parts = [res.results[i]["out"] for i in range(8)]
    out = combine(parts)
    if _trace:
        return out, res
    return out



# revision 9
# speedup vs baseline: 1.4440x; 1.4440x over previous
"""DetectionBEVLoss Trainium2 kernel: 8-core data-parallel (1 batch/core).

Per core 65536 elements as [128 partitions, 512 free], full-width ops.
Rotated IoU via branch-free Liang-Barsky edge clipping with closed-form
edge directions (edge dirs of a rotated rect are +-2*{cos,sin}(dyaw)*halfdim,
so the clip reciprocals come straight from the trig products - no corner
differencing) and a closed-form A-side cross-product sum:
  sum_e CR_e*S_e = 2*lhp*dcy2*(S0-S2) + 2*whp*dcx2*(S1-S3) - 2*lhp*whp*sum(S_A)
Engines: DVE does the TT-heavy geometry; ACT (single table set:
natural_log_exp_and_others = abs/relu/square/exp/ln/copy) takes the
activations; gpsimd is kept off the critical path. All 9 loss partial sums
are fused multiply+reduce (tensor_tensor_reduce) into one fp32 accumulator,
cross-partition reduced by one TensorE matmul.
Input DMA is staged in 3 pieces (geometry slots first) so compute starts
~4us in instead of waiting for the full 4MiB.
"""
import numpy as np

import concourse.bacc as bacc
import concourse.bass as bass
import concourse.mybir as mybir
import concourse.tile as tile
from concourse.bass_utils import run_bass_kernel_spmd

F16 = mybir.dt.float16
F32 = mybir.dt.float32
OP = mybir.AluOpType
AF = mybir.ActivationFunctionType

P = 128
FW = 512
EPS = 1e-7

# IN1 slots: 0 yawp, 1 yawt, 2 wp, 3 lp, 4 wt, 5 lt, 6 xp, 7 yp, 8 xt, 9 yt
# IN2 slots: 0 zp, 1 zt, 2 hp, 3 ht, 4 vxp, 5 vxt, 6 vyp, 7 vyt,
#            8 ioup, 9 iout, 10 clst, 11 w
# IN3 slots: cls_pred c0..c9


def _ap(t, s0, slot_dims, col0=0, ncol=FW, colstep=1):
    """Manual AP into tile t ([128, S, W]): base slot s0, then
    (slot_step, count) dims, innermost column dim."""
    ss = t.ap[-2][0]
    ap = [list(t.ap[0])] + [[s * ss, c] for s, c in slot_dims] + [[colstep, ncol]]
    return bass.AP(tensor=t.tensor, offset=t.offset + s0 * ss + col0, ap=ap)


DBG_SLOTS = 64


def build_bass(dbg=False):
    nc = bacc.Bacc("TRN2", target_bir_lowering=False, debug=False)
    in1 = nc.declare_dram_parameter("in1", [P, 10, FW], F16, isOutput=False)
    in2 = nc.declare_dram_parameter("in2", [P, 12, FW], F16, isOutput=False)
    in3 = nc.declare_dram_parameter("in3", [P, 10, FW], F16, isOutput=False)
    outp = nc.declare_dram_parameter("out", [1, 16], F32, isOutput=True)
    dbg_slots = {}
    if dbg:
        dbgp = nc.declare_dram_parameter("dbg", [P, DBG_SLOTS, FW], F16,
                                         isOutput=True)
        dbg_next = [0]

        def tap(name, t, k):
            s = dbg_next[0]
            assert s + k <= DBG_SLOTS
            nc.sync.dma_start(out=dbgp[:, s:s + k, :], in_=t)
            dbg_slots[name] = (s, k)
            dbg_next[0] += k
    else:
        def tap(name, t, k):
            pass

    with tile.TileContext(nc) as tc:
        with (
            tc.tile_pool(name="main", bufs=1) as pool,
            tc.tile_pool(name="small", bufs=1) as spool,
            tc.tile_pool(name="ps", bufs=1, space="PSUM") as ppool,
        ):
            IN1 = pool.tile([P, 10, FW], F16)
            IN2 = pool.tile([P, 12, FW], F16)
            IN3 = pool.tile([P, 10, FW], F16)
            nc.sync.dma_start(out=IN1, in_=in1[:, :, :])
            nc.sync.dma_start(out=IN2, in_=in2[:, :, :])
            nc.sync.dma_start(out=IN3, in_=in3[:, :, :])

            ones = spool.tile([P, 1], F32)
            nc.vector.memset(ones, 1.0)
            ACC = spool.tile([P, 16], F32)
            nc.vector.memset(ACC, 0.0)
            JUNK = pool.tile([P, FW], F16, tag="JUNK")

            def acc_sum(in0, in1_, col, scale=1.0, out=None):
                # fused (in0*scale)*in1 with free-dim sum into ACC[:, col]
                nc.vector.scalar_tensor_tensor(
                    out=out if out is not None else JUNK,
                    in0=in0, scalar=scale, in1=in1_,
                    op0=OP.mult, op1=OP.mult,
                    accum_out=ACC[:, col:col + 1])

            # ================= trig (needs IN1 only) =================
            HV = pool.tile([P, 4, FW], F16)       # [lht, wht, lhp, whp]
            nc.vector.tensor_scalar(out=HV, in0=_ap(IN1, 5, [(-1, 4)]),
                                    scalar1=0.5, scalar2=None, op0=OP.mult)
            DXY = pool.tile([P, 2, FW], F16)      # [dx, dy]
            nc.vector.tensor_tensor(out=DXY, in0=IN1[:, 6:8, :],
                                    in1=IN1[:, 8:10, :], op=OP.subtract)

            X2 = pool.tile([P, 2, FW], F16, tag="T2a")
            nc.scalar.activation(X2, IN1[:, 0:2, :], AF.Square)
            TR = pool.tile([P, 4, FW], F16)       # [cp, sp, ct, st]
            SPH = pool.tile([P, 2, FW], F16, tag="T2b")
            nc.vector.tensor_scalar(out=SPH, in0=X2, scalar1=1.0 / 120,
                                    scalar2=-1.0 / 6, op0=OP.mult, op1=OP.add)
            nc.vector.tensor_tensor(out=SPH, in0=SPH, in1=X2, op=OP.mult)
            nc.vector.scalar_tensor_tensor(out=_ap(TR, 1, [(2, 2)]), in0=SPH,
                                           scalar=1.0, in1=IN1[:, 0:2, :],
                                           op0=OP.add, op1=OP.mult)
            CPH = pool.tile([P, 2, FW], F16, tag="T2c")
            nc.vector.tensor_scalar(out=CPH, in0=X2, scalar1=-1.0 / 720,
                                    scalar2=1.0 / 24, op0=OP.mult, op1=OP.add)
            nc.vector.tensor_tensor(out=CPH, in0=CPH, in1=X2, op=OP.mult)
            nc.vector.tensor_scalar(out=CPH, in0=CPH, scalar1=-0.5,
                                    scalar2=None, op0=OP.add)
            nc.vector.tensor_tensor(out=CPH, in0=CPH, in1=X2, op=OP.mult)
            nc.vector.tensor_scalar(out=_ap(TR, 0, [(2, 2)]), in0=CPH,
                                    scalar1=1.0, scalar2=None, op0=OP.add)
            tap("TR", TR, 4)
            ACS = pool.tile([P, 4, FW], F16)      # |cp| |sp| |ct| |st|
            nc.scalar.activation(ACS, TR, AF.Abs)

            tap("HV", HV, 4)
            tap("DXY", DXY, 2)
            # TP=[cp*ct, sp*st], TQ=[sp*ct, cp*st]
            TP = pool.tile([P, 2, FW], F16, tag="T2a")
            TQ = pool.tile([P, 2, FW], F16, tag="T2b")
            nc.vector.tensor_tensor(out=TP, in0=TR[:, 0:2, :], in1=TR[:, 2:4, :],
                                    op=OP.mult)
            nc.vector.tensor_tensor(out=TQ, in0=_ap(TR, 1, [(-1, 2)]),
                                    in1=TR[:, 2:4, :], op=OP.mult)
            # CS8 = [-cd,-sd,-sd,+cd,+cd,-sd,-sd,-cd]; cd=TP0+TP1, sd=TQ0-TQ1
            CS8 = pool.tile([P, 8, FW], F16, tag="S8a")
            nc.vector.tensor_tensor(out=_ap(CS8, 3, [(1, 2)]),
                                    in0=_ap(TP, 0, [(0, 2)]),
                                    in1=_ap(TP, 1, [(0, 2)]), op=OP.add)
            nc.vector.scalar_tensor_tensor(out=_ap(CS8, 0, [(7, 2)]),
                                           in0=_ap(TP, 0, [(0, 2)]), scalar=-1.0,
                                           in1=_ap(TP, 1, [(0, 2)]),
                                           op0=OP.mult, op1=OP.subtract)
            nc.vector.tensor_tensor(out=_ap(CS8, 1, [(4, 2), (1, 2)]),
                                    in0=_ap(TQ, 1, [(0, 2), (0, 2)]),
                                    in1=_ap(TQ, 0, [(0, 2), (0, 2)]),
                                    op=OP.subtract)

            # DC = [dcx, dcy, dcx2, dcy2]
            PT = pool.tile([P, 4, FW], F16, tag="S4a")
            QT = pool.tile([P, 4, FW], F16, tag="S4b")
            nc.vector.tensor_tensor(out=PT, in0=_ap(DXY, 0, [(0, 2), (1, 2)]),
                                    in1=_ap(TR, 2, [(-2, 2), (0, 2)]), op=OP.mult)
            nc.vector.tensor_tensor(out=QT, in0=_ap(DXY, 0, [(0, 2), (1, 2)]),
                                    in1=_ap(TR, 3, [(-2, 2), (0, 2)]), op=OP.mult)
            DC = pool.tile([P, 4, FW], F16)
            nc.vector.tensor_tensor(out=_ap(DC, 0, [(2, 2)]),
                                    in0=_ap(PT, 0, [(2, 2)]),
                                    in1=_ap(QT, 1, [(2, 2)]), op=OP.add)
            nc.vector.tensor_tensor(out=_ap(DC, 1, [(2, 2)]),
                                    in0=_ap(PT, 1, [(2, 2)]),
                                    in1=_ap(QT, 0, [(2, 2)]), op=OP.subtract)

            tap("DC", DC, 4)
            # UVXD = CS8 * [lhp,whp,lhp,whp,lht,wht,lht,wht]  (= D/2)
            UVXD = pool.tile([P, 8, FW], F16, tag="S8b")
            nc.vector.tensor_tensor(out=UVXD, in0=CS8,
                                    in1=_ap(HV, 2, [(-2, 2), (0, 2), (1, 2)]),
                                    op=OP.mult)

            tap("UVXD", UVXD, 8)
            # SC8 = [sA, sC, sB, sD, pA, pB, pC, pD]
            SC8 = pool.tile([P, 8, FW], F16, tag="S8c")
            nc.vector.scalar_tensor_tensor(out=_ap(SC8, 0, [(1, 2)]),
                                           in0=_ap(UVXD, 0, [(2, 2)]), scalar=-1.0,
                                           in1=_ap(UVXD, 1, [(2, 2)]),
                                           op0=OP.mult, op1=OP.subtract)
            nc.vector.tensor_tensor(out=_ap(SC8, 2, [(1, 2)]),
                                    in0=_ap(UVXD, 1, [(2, 2)]),
                                    in1=_ap(UVXD, 0, [(2, 2)]), op=OP.subtract)
            nc.vector.tensor_tensor(out=SC8[:, 4, :], in0=UVXD[:, 4, :],
                                    in1=UVXD[:, 5, :], op=OP.add)
            nc.vector.tensor_tensor(out=SC8[:, 5, :], in0=UVXD[:, 4, :],
                                    in1=UVXD[:, 5, :], op=OP.subtract)
            nc.vector.scalar_tensor_tensor(out=SC8[:, 6, :], in0=UVXD[:, 6, :],
                                           scalar=-1.0, in1=UVXD[:, 7, :],
                                           op0=OP.mult, op1=OP.subtract)
            nc.vector.tensor_tensor(out=SC8[:, 7, :], in0=UVXD[:, 7, :],
                                    in1=UVXD[:, 6, :], op=OP.subtract)

            tap("SC8", SC8, 8)
            # corners: CRN = [AX0..3, AY0..3, BX0..3, BY0..3]
            CRN = pool.tile([P, 16, FW], F16, tag="B16a")

            def corner2(dst0, dstep, dcslot, scslot, scstep, op):
                nc.vector.tensor_tensor(
                    out=_ap(CRN, dst0, [(dstep, 2)]),
                    in0=_ap(DC, dcslot, [(0, 2)]),
                    in1=_ap(SC8, scslot, [(scstep, 2)]), op=op)

            corner2(0, 3, 0, 0, 2, OP.add)        # AX0=dcx+sA, AX3=dcx+sB
            corner2(1, 1, 0, 2, -2, OP.subtract)  # AX1=dcx-sB, AX2=dcx-sA
            corner2(4, 3, 1, 1, 2, OP.add)        # AY0=dcy+sC, AY3=dcy+sD
            corner2(5, 1, 1, 3, -2, OP.subtract)  # AY1=dcy-sD, AY2=dcy-sC
            corner2(8, 3, 2, 4, 1, OP.subtract)   # BX0=dcx2-pA, BX3=dcx2-pB
            corner2(9, 1, 2, 5, -1, OP.add)       # BX1=dcx2+pB, BX2=dcx2+pA
            corner2(12, 3, 3, 6, 1, OP.add)       # BY0=dcy2+pC, BY3=dcy2+pD
            corner2(13, 1, 3, 7, -1, OP.subtract) # BY1=dcy2-pD, BY2=dcy2-pC

            # ============ clip: reciprocals from UVXD ============
            UVX32 = pool.tile([P, 8, FW], F32, tag="F32a")
            nc.scalar.activation(UVX32, UVXD, AF.Copy, bias=1e-12)
            REC32 = UVX32
            nc.vector.reciprocal_approx_fast(
                out=REC32.rearrange("p a b -> p (a b)"),
                in_=UVX32.rearrange("p a b -> p (a b)"))
            # r = REC/2 clamped to +-8000
            nc.vector.tensor_scalar(out=REC32, in0=REC32, scalar1=0.5,
                                    scalar2=-8000.0, op0=OP.mult, op1=OP.max)
            RD8 = pool.tile([P, 8, FW], F16, tag="S8b")
            nc.vector.tensor_scalar(out=RD8, in0=REC32, scalar1=8000.0,
                                    scalar2=None, op0=OP.min)
            tap("RD8", RD8, 8)
            RA = pool.tile([P, 8, FW], F16, tag="S8a")
            nc.scalar.activation(RA, RD8, AF.Abs)
            RL = pool.tile([P, 8, FW], F16, tag="S8c")
            nc.vector.tensor_tensor(out=RL, in0=RA,
                                    in1=_ap(HV, 0, [(1, 4), (0, 2)]), op=OP.mult)

            tap("RL", RL, 8)
            rep16 = [(2, 4), (0, 2), (1, 2)]
            P16 = pool.tile([P, 16, FW], F16, tag="B16b")
            nc.vector.tensor_tensor(out=P16, in0=CRN,
                                    in1=_ap(RD8, 0, rep16), op=OP.mult)
            OPA = pool.tile([P, 16, FW], F16, tag="B16a")   # reuse CRN buffer
            nc.vector.tensor_tensor(out=OPA, in0=_ap(RL, 0, rep16), in1=P16,
                                    op=OP.subtract)
            OPB = P16   # in-place: OPB = RL16rep + P16 overwrites P16
            nc.vector.tensor_tensor(out=OPB, in0=_ap(RL, 0, rep16), in1=P16,
                                    op=OP.add)

            # T1m/T0m: min over the two axes; edges {0,1} vs {2,3} swap A/B roles
            T1m = pool.tile([P, 8, FW], F16, tag="S8a")
            T0m = pool.tile([P, 8, FW], F16, tag="S8b")
            nc.vector.tensor_tensor(out=_ap(T1m, 0, [(4, 2), (1, 2)]),
                                    in0=_ap(OPA, 0, [(8, 2), (1, 2)]),
                                    in1=_ap(OPA, 4, [(8, 2), (1, 2)]), op=OP.min)
            nc.vector.tensor_tensor(out=_ap(T1m, 2, [(4, 2), (1, 2)]),
                                    in0=_ap(OPB, 2, [(8, 2), (1, 2)]),
                                    in1=_ap(OPB, 6, [(8, 2), (1, 2)]), op=OP.min)
            nc.vector.tensor_tensor(out=_ap(T0m, 0, [(4, 2), (1, 2)]),
                                    in0=_ap(OPB, 0, [(8, 2), (1, 2)]),
                                    in1=_ap(OPB, 4, [(8, 2), (1, 2)]), op=OP.min)
            nc.vector.tensor_tensor(out=_ap(T0m, 2, [(4, 2), (1, 2)]),
                                    in0=_ap(OPA, 2, [(8, 2), (1, 2)]),
                                    in1=_ap(OPA, 6, [(8, 2), (1, 2)]), op=OP.min)
            # SEG = relu(min(T1,1) - relu(-T0m))
            nc.scalar.activation(T0m, T0m, AF.Relu, scale=-1.0)
            nc.vector.tensor_scalar(out=T1m, in0=T1m, scalar1=1.0,
                                    scalar2=None, op0=OP.min)
            SEG = pool.tile([P, 8, FW], F16, tag="S8c")
            nc.vector.tensor_tensor(out=SEG, in0=T1m, in1=T0m, op=OP.subtract)
            nc.scalar.activation(SEG, SEG, AF.Relu)

            tap("SEG", SEG, 8)
            # ============ intersection (closed-form cross sum) ============
            SD1 = pool.tile([P, 2, FW], F16, tag="T2a")   # [S0-S2, S1-S3]
            nc.vector.tensor_tensor(out=SD1, in0=_ap(SEG, 0, [(1, 2)]),
                                    in1=_ap(SEG, 2, [(1, 2)]), op=OP.subtract)
            SALL = pool.tile([P, 4, FW], F16, tag="S4a")  # [S0+S2,S1+S3,S4+S6,S5+S7]
            nc.vector.tensor_tensor(out=SALL, in0=_ap(SEG, 0, [(4, 2), (1, 2)]),
                                    in1=_ap(SEG, 2, [(4, 2), (1, 2)]), op=OP.add)
            SS2 = pool.tile([P, 2, FW], F16, tag="T2b")   # [sumS_A, sumS_B]
            nc.vector.tensor_tensor(out=SS2, in0=_ap(SALL, 0, [(2, 2)]),
                                    in1=_ap(SALL, 1, [(2, 2)]), op=OP.add)
            Pm = pool.tile([P, 2, FW], F16, tag="T2c")    # [dcy2*SD0, dcx2*SD1]
            nc.vector.tensor_tensor(out=Pm, in0=SD1,
                                    in1=_ap(DC, 3, [(-1, 2)]), op=OP.mult)
            nc.vector.tensor_tensor(out=Pm, in0=Pm,
                                    in1=_ap(HV, 2, [(1, 2)]), op=OP.mult)
            AREA2 = pool.tile([P, 2, FW], F16, tag="A2")  # [lhp*whp, lht*wht]
            nc.vector.tensor_tensor(out=AREA2, in0=_ap(HV, 2, [(-2, 2)]),
                                    in1=_ap(HV, 3, [(-2, 2)]), op=OP.mult)
            MM2 = pool.tile([P, 2, FW], F16, tag="T2d")
            nc.vector.tensor_tensor(out=MM2, in0=AREA2, in1=SS2, op=OP.mult)
            HACA = pool.tile([P, FW], F16, tag="K1")
            nc.vector.tensor_tensor(out=HACA, in0=Pm[:, 0, :], in1=Pm[:, 1, :],
                                    op=OP.add)
            nc.vector.tensor_tensor(out=HACA, in0=HACA, in1=MM2[:, 0, :],
                                    op=OP.subtract)
            nc.vector.tensor_tensor(out=HACA, in0=HACA, in1=MM2[:, 1, :],
                                    op=OP.subtract)
            INTER = pool.tile([P, FW], F16, tag="K2")
            nc.scalar.activation(INTER, HACA, AF.Abs)

            tap("INTER", INTER, 1)
            U1 = pool.tile([P, FW], F16, tag="K3")
            nc.vector.tensor_tensor(out=U1, in0=AREA2[:, 0, :],
                                    in1=AREA2[:, 1, :], op=OP.add)
            UNION = pool.tile([P, FW], F16, tag="K4")
            nc.vector.scalar_tensor_tensor(out=UNION, in0=U1, scalar=4.0,
                                           in1=INTER, op0=OP.mult, op1=OP.subtract)
            MU = pool.tile([P, FW], F16, tag="K5")
            nc.vector.tensor_scalar(out=MU, in0=UNION, scalar1=EPS,
                                    scalar2=None, op0=OP.is_gt)
            ING = INTER
            nc.vector.tensor_tensor(out=ING, in0=INTER, in1=MU, op=OP.mult)
            UC = UNION
            nc.vector.tensor_scalar(out=UC, in0=UNION, scalar1=EPS,
                                    scalar2=None, op0=OP.max)

            # ============ enclosing box + center dist ============
            PA_ = pool.tile([P, 4, FW], F16, tag="S4a")
            PB_ = pool.tile([P, 4, FW], F16, tag="S4b")
            nc.vector.tensor_tensor(out=PA_, in0=_ap(HV, 2, [(-2, 2), (1, 2)]),
                                    in1=ACS, op=OP.mult)
            nc.vector.tensor_tensor(out=PB_, in0=_ap(HV, 2, [(-2, 2), (1, 2)]),
                                    in1=_ap(ACS, 1, [(2, 2), (-1, 2)]), op=OP.mult)
            E2 = pool.tile([P, 4, FW], F16, tag="S4c")    # [exP, exT, eyP, eyT]
            nc.vector.tensor_tensor(out=_ap(E2, 0, [(1, 2)]),
                                    in0=_ap(PA_, 0, [(2, 2)]),
                                    in1=_ap(PA_, 1, [(2, 2)]), op=OP.add)
            nc.vector.tensor_tensor(out=_ap(E2, 2, [(1, 2)]),
                                    in0=_ap(PB_, 0, [(2, 2)]),
                                    in1=_ap(PB_, 1, [(2, 2)]), op=OP.add)
            # Earr = [exP, eyP, exT, eyT]; CEN = [xp, yp, xt, yt]
            XE = pool.tile([P, 4, FW], F16, tag="S4d")
            XD = pool.tile([P, 4, FW], F16, tag="S4e")
            # Earr: slots (0,2,1,3) of E2 => [exP, eyP, exT, eyT]
            Earr = _ap(E2, 0, [(1, 2), (2, 2)])
            nc.vector.tensor_tensor(out=XE, in0=IN1[:, 6:10, :], in1=Earr,
                                    op=OP.add)
            nc.vector.tensor_tensor(out=XD, in0=IN1[:, 6:10, :], in1=Earr,
                                    op=OP.subtract)
            HX = pool.tile([P, 2, FW], F16, tag="T2a")
            LX = pool.tile([P, 2, FW], F16, tag="T2b")
            nc.vector.tensor_tensor(out=HX, in0=_ap(XE, 0, [(1, 2)]),
                                    in1=_ap(XE, 2, [(1, 2)]), op=OP.max)
            nc.vector.tensor_tensor(out=LX, in0=_ap(XD, 0, [(1, 2)]),
                                    in1=_ap(XD, 2, [(1, 2)]), op=OP.min)
            W2 = pool.tile([P, 2, FW], F16, tag="T2c")
            nc.vector.tensor_tensor(out=W2, in0=HX, in1=LX, op=OP.subtract)
            SQ2 = pool.tile([P, 2, FW], F16, tag="T2d")
            nc.scalar.activation(SQ2, W2, AF.Square)
            C2 = pool.tile([P, FW], F16, tag="K8")
            nc.vector.tensor_tensor(out=C2, in0=SQ2[:, 0, :], in1=SQ2[:, 1, :],
                                    op=OP.add)
            nc.vector.tensor_scalar(out=C2, in0=C2, scalar1=EPS,
                                    scalar2=None, op0=OP.max)
            D2P = pool.tile([P, 2, FW], F16, tag="T2e")
            nc.scalar.activation(D2P, DXY, AF.Square)
            D2 = pool.tile([P, FW], F16, tag="K9")
            nc.vector.tensor_tensor(out=D2, in0=D2P[:, 0, :], in1=D2P[:, 1, :],
                                    op=OP.add)

            # DL = (d2*UC - ING*C2) / (C2*UC); one reciprocal
            CM = pool.tile([P, FW], F32, tag="KF1")
            nc.vector.tensor_tensor(out=CM, in0=C2, in1=UC, op=OP.mult)
            RECM = pool.tile([P, FW], F32, tag="KF2")
            nc.vector.reciprocal_approx_fast(out=RECM, in_=CM)
            N1 = D2
            nc.vector.tensor_tensor(out=N1, in0=D2, in1=UC, op=OP.mult)
            N2 = pool.tile([P, FW], F16, tag="K11")
            nc.vector.tensor_tensor(out=N2, in0=ING, in1=C2, op=OP.mult)
            nc.vector.tensor_tensor(out=N1, in0=N1, in1=N2, op=OP.subtract)
            DL = N1
            nc.vector.tensor_tensor(out=DL, in0=N1, in1=RECM, op=OP.mult)
            wm = IN2[:, 11, :]
            acc_sum(DL, wm, 2)

            tap("C2", C2, 1)
            tap("D2", D2, 1)
            tap("DL", DL, 1)
            # ============ smooth L1 (z, h, vx, vy) ============
            DD = pool.tile([P, 4, FW], F16, tag="S4a")
            nc.vector.tensor_tensor(out=DD, in0=_ap(IN2, 0, [(2, 4)]),
                                    in1=_ap(IN2, 1, [(2, 4)]), op=OP.subtract)
            AD = pool.tile([P, 4, FW], F16, tag="S4b")
            nc.scalar.activation(AD, DD, AF.Abs)
            RM = pool.tile([P, 4, FW], F16, tag="S4c")
            nc.scalar.activation(RM, AD, AF.Relu, scale=-1.0, bias=1.0)
            R2 = pool.tile([P, 4, FW], F16, tag="S4d")
            nc.vector.tensor_tensor(out=R2, in0=RM, in1=RM, op=OP.mult)
            SL = pool.tile([P, 4, FW], F16, tag="S4e")
            nc.vector.scalar_tensor_tensor(out=SL, in0=R2, scalar=0.5, in1=AD,
                                           op0=OP.mult, op1=OP.add)
            for k in range(4):
                acc_sum(SL[:, k, :], wm, 3 + k)

            # ============ BCE on iou head ============
            iop = IN2[:, 8, :]
            BA = pool.tile([P, FW], F16, tag="K13")
            nc.scalar.activation(BA, iop, AF.Abs)
            nc.scalar.activation(BA, BA, AF.Exp, scale=-1.0)
            nc.scalar.activation(BA, BA, AF.Ln, bias=1.0)
            BR = pool.tile([P, FW], F16, tag="K14")
            nc.scalar.activation(BR, iop, AF.Relu)
            BXY = pool.tile([P, FW], F16, tag="K15")
            nc.vector.tensor_tensor(out=BXY, in0=iop, in1=IN2[:, 9, :],
                                    op=OP.mult)
            nc.vector.tensor_tensor(out=BR, in0=BR, in1=BXY, op=OP.subtract)
            nc.vector.tensor_tensor(out=BR, in0=BR, in1=BA, op=OP.add)
            acc_sum(BR, wm, 7)
            acc_sum(wm, wm, 8)

            # ============ focal ============
            clsf = IN2[:, 10, :]
            ET = pool.tile([P, 10, FW], F16, tag="S10a")
            nc.scalar.activation(ET, IN3, AF.Exp)
            S5 = pool.tile([P, 5, FW], F16, tag="S5a")
            nc.vector.tensor_tensor(out=S5, in0=ET[:, 0:5, :], in1=ET[:, 5:10, :],
                                    op=OP.add)
            S2 = pool.tile([P, 2, FW], F16, tag="T2a")
            nc.vector.tensor_tensor(out=S2, in0=S5[:, 0:2, :], in1=S5[:, 2:4, :],
                                    op=OP.add)
            SSs = pool.tile([P, FW], F16, tag="K16")
            nc.vector.tensor_tensor(out=SSs, in0=S2[:, 0, :], in1=S2[:, 1, :],
                                    op=OP.add)
            nc.vector.tensor_tensor(out=SSs, in0=SSs, in1=S5[:, 4, :], op=OP.add)
            MT = pool.tile([P, 10, FW], F16, tag="S10a")   # reuse ET buffer
            for c in range(10):
                nc.vector.scalar_tensor_tensor(out=MT[:, c, :], in0=clsf,
                                               scalar=float(c), in1=IN3[:, c, :],
                                               op0=OP.is_equal, op1=OP.mult)
            L5 = pool.tile([P, 5, FW], F16, tag="S5a")
            nc.vector.tensor_tensor(out=L5, in0=MT[:, 0:5, :], in1=MT[:, 5:10, :],
                                    op=OP.add)
            L2 = pool.tile([P, 2, FW], F16, tag="T2b")
            nc.vector.tensor_tensor(out=L2, in0=L5[:, 0:2, :], in1=L5[:, 2:4, :],
                                    op=OP.add)
            LT = pool.tile([P, FW], F16, tag="K17")
            nc.vector.tensor_tensor(out=LT, in0=L2[:, 0, :], in1=L2[:, 1, :],
                                    op=OP.add)
            nc.vector.tensor_tensor(out=LT, in0=LT, in1=L5[:, 4, :], op=OP.add)
            LNS = SSs
            nc.scalar.activation(LNS, SSs, AF.Ln)
            LPT = LT
            nc.vector.tensor_tensor(out=LPT, in0=LT, in1=LNS, op=OP.subtract)
            PTT = pool.tile([P, FW], F16, tag="K20")
            nc.scalar.activation(PTT, LPT, AF.Exp)
            OM2 = PTT
            nc.scalar.activation(OM2, PTT, AF.Square, scale=-1.0, bias=1.0)
            F1 = OM2
            nc.vector.tensor_tensor(out=F1, in0=OM2, in1=LPT, op=OP.mult)
            MPOS = pool.tile([P, FW], F16, tag="K23")
            nc.vector.tensor_scalar(out=MPOS, in0=clsf, scalar1=0.5,
                                    scalar2=None, op0=OP.is_gt)
            nc.vector.tensor_scalar(out=MPOS, in0=MPOS, scalar1=-0.5,
                                    scalar2=0.75, op0=OP.mult, op1=OP.add)
            nc.vector.tensor_tensor(out=F1, in0=F1, in1=MPOS, op=OP.mult)
            VLD = pool.tile([P, FW], F16, tag="K24")
            nc.vector.tensor_scalar(out=VLD, in0=clsf, scalar1=-0.5,
                                    scalar2=None, op0=OP.is_ge)
            acc_sum(F1, VLD, 0, scale=-1.0)
            acc_sum(VLD, VLD, 1)

            # ============ cross-partition reduce + output ============
            PS = ppool.tile([1, 16], F32)
            nc.tensor.matmul(PS, ones, ACC, start=True, stop=True)
            OUT = spool.tile([1, 16], F32)
            nc.scalar.copy(out=OUT, in_=PS)
            nc.sync.dma_start(out=outp[:, :], in_=OUT)
    nc.compile()
    nc._dbg_slots = dbg_slots
    return nc


_NC_CACHE = None


def _get_nc():
    global _NC_CACHE
    if _NC_CACHE is None:
        _NC_CACHE = build_bass()
    return _NC_CACHE


def pack_inputs(cls_pred, reg_pred, iou_pred, reg_targets, iou_targets,
                cls_targets, reg_weights):
    """Returns list of 8 per-core input dicts (in1/in2/in3 fp16 arrays)."""
    B = cls_pred.shape[0]
    maps = []
    for b in range(B):
        rp = np.asarray(reg_pred[b], np.float32).reshape(9, P, FW)
        rt = np.asarray(reg_targets[b], np.float32).reshape(9, P, FW)
        h1 = np.empty((10, P, FW), np.float16)
        h1[0] = rp[6]; h1[1] = rt[6]
        h1[2] = rp[3]; h1[3] = rp[4]
        h1[4] = rt[3]; h1[5] = rt[4]
        h1[6] = rp[0]; h1[7] = rp[1]
        h1[8] = rt[0]; h1[9] = rt[1]
        h2 = np.empty((12, P, FW), np.float16)
        h2[0] = rp[2]; h2[1] = rt[2]
        h2[2] = rp[5]; h2[3] = rt[5]
        h2[4] = rp[7]; h2[5] = rt[7]
        h2[6] = rp[8]; h2[7] = rt[8]
        h2[8] = np.asarray(iou_pred[b], np.float32).reshape(P, FW)
        h2[9] = np.asarray(iou_targets[b], np.float32).reshape(P, FW)
        h2[10] = np.asarray(cls_targets[b]).astype(np.float32).reshape(P, FW)
        h2[11] = np.asarray(reg_weights[b]).astype(np.float32).reshape(P, FW)
        h3 = np.asarray(cls_pred[b], np.float32).reshape(10, P, FW).astype(np.float16)
        maps.append({
            "in1": np.ascontiguousarray(h1.transpose(1, 0, 2)),
            "in2": np.ascontiguousarray(h2.transpose(1, 0, 2)),
            "in3": np.ascontiguousarray(h3.transpose(1, 0, 2)),
        })
    return maps


def combine(parts):
    """parts: [8, 1, 16] per-core raw sums -> final [7] float32."""
    p = np.asarray(parts, np.float64).sum(0).reshape(-1)
    focal_s, valid_s, diou_s, z_s, h_s, vx_s, vy_s, bce_s, w_s = p[:9]
    num_pos = max(w_s, 1.0)
    cls_loss = focal_s / max(valid_s, 1.0)
    bev_loss = (diou_s + w_s) / num_pos
    z_loss = (z_s - 0.5 * w_s) / num_pos
    h_loss = (h_s - 0.5 * w_s) / num_pos
    vel_loss = (vx_s + vy_s - w_s) / num_pos
    iou_loss = bce_s / num_pos
    total = cls_loss + 2.0 * bev_loss + z_loss + h_loss + vel_loss + iou_loss
    return np.array([total, cls_loss, bev_loss, z_loss, h_loss, vel_loss, iou_loss],
                    np.float32)


def kernel(cls_pred, reg_pred, iou_pred, reg_targets, iou_targets,
           cls_targets, reg_weights, _trace=False):
    cls_pred, reg_pred, iou_pred, reg_targets, iou_targets, cls_targets, reg_weights = (
        np.asarray(a) for a in (cls_pred, reg_pred, iou_pred, reg_targets,
                                iou_targets, cls_targets, reg_weights))
    nc = _get_nc()
    in_maps = pack_inputs(cls_pred, reg_pred, iou_pred, reg_targets,
                          iou_targets, cls_targets, reg_weights)
    res = run_bass_kernel_spmd(nc, in_maps, core_ids=list(range(8)), trace=_trace)
    parts = [res.results[i]["out"] for i in range(8)]
    out = combine(parts)
    if _trace:
        return out, res
    return out


# revision 11
# speedup vs baseline: 1.6923x; 1.1719x over previous
"""DetectionBEVLoss Trainium2 kernel: 8-core data-parallel (1 batch/core).

Per core 65536 elements as [128 partitions, 512 free], full-width ops.
Rotated IoU via branch-free Liang-Barsky edge clipping with closed-form
edge directions (edge dirs of a rotated rect are +-2*{cos,sin}(dyaw)*halfdim,
so the clip reciprocals come straight from the trig products - no corner
differencing) and a closed-form A-side cross-product sum:
  sum_e CR_e*S_e = 2*lhp*dcy2*(S0-S2) + 2*whp*dcx2*(S1-S3) - 2*lhp*whp*sum(S_A)
Engines: DVE does the TT-heavy geometry; ACT (single table set:
natural_log_exp_and_others = abs/relu/square/exp/ln/copy) takes the
activations; gpsimd is kept off the critical path. All 9 loss partial sums
are fused multiply+reduce (tensor_tensor_reduce) into one fp32 accumulator,
cross-partition reduced by one TensorE matmul.
Input DMA is staged in 3 pieces (geometry slots first) so compute starts
~4us in instead of waiting for the full 4MiB.
"""
import numpy as np

import concourse.bacc as bacc
import concourse.bass as bass
import concourse.mybir as mybir
import concourse.tile as tile
from concourse.bass_utils import run_bass_kernel_spmd

F16 = mybir.dt.float16
F32 = mybir.dt.float32
OP = mybir.AluOpType
AF = mybir.ActivationFunctionType

P = 128
FW = 512
EPS = 1e-7

# IN1 slots: 0 yawp, 1 yawt, 2 wp, 3 lp, 4 wt, 5 lt, 6 xp, 7 yp, 8 xt, 9 yt
# IN2 slots: 0 zp, 1 zt, 2 hp, 3 ht, 4 vxp, 5 vxt, 6 vyp, 7 vyt,
#            8 ioup, 9 iout, 10 clst, 11 w
# IN3 slots: cls_pred c0..c9


def _ap(t, s0, slot_dims, col0=0, ncol=FW, colstep=1):
    """Manual AP into tile t ([128, S, W]): base slot s0, then
    (slot_step, count) dims, innermost column dim."""
    ss = t.ap[-2][0]
    ap = [list(t.ap[0])] + [[s * ss, c] for s, c in slot_dims] + [[colstep, ncol]]
    return bass.AP(tensor=t.tensor, offset=t.offset + s0 * ss + col0, ap=ap)


DBG_SLOTS = 64


def build_bass(dbg=False):
    nc = bacc.Bacc("TRN2", target_bir_lowering=False, debug=False)
    in1 = nc.declare_dram_parameter("in1", [P, 10, FW], F16, isOutput=False)
    in2 = nc.declare_dram_parameter("in2", [P, 12, FW], F16, isOutput=False)
    in3 = nc.declare_dram_parameter("in3", [P, 10, FW], F16, isOutput=False)
    outp = nc.declare_dram_parameter("out", [1, 16], F32, isOutput=True)
    dbg_slots = {}
    if dbg:
        dbgp = nc.declare_dram_parameter("dbg", [P, DBG_SLOTS, FW], F16,
                                         isOutput=True)
        dbg_next = [0]

        def tap(name, t, k):
            s = dbg_next[0]
            assert s + k <= DBG_SLOTS
            nc.sync.dma_start(out=dbgp[:, s:s + k, :], in_=t)
            dbg_slots[name] = (s, k)
            dbg_next[0] += k
    else:
        def tap(name, t, k):
            pass

    with tile.TileContext(nc) as tc:
        with (
            tc.tile_pool(name="main", bufs=1) as pool,
            tc.tile_pool(name="small", bufs=1) as spool,
            tc.tile_pool(name="ps", bufs=1, space="PSUM") as ppool,
        ):
            IN1 = pool.tile([P, 10, FW], F16)
            IN2 = pool.tile([P, 12, FW], F16)
            IN3 = pool.tile([P, 10, FW], F16)
            nc.sync.dma_start(out=IN1, in_=in1[:, :, :])
            nc.sync.dma_start(out=IN2, in_=in2[:, :, :])
            nc.sync.dma_start(out=IN3, in_=in3[:, :, :])

            ones = spool.tile([P, 1], F32)
            nc.vector.memset(ones, 1.0)
            ACC = spool.tile([P, 16], F32)
            nc.vector.memset(ACC, 0.0)
            JUNK = pool.tile([P, FW], F16, tag="JUNK")

            def acc_sum(in0, in1_, col, scale=1.0, out=None):
                # fused (in0*scale)*in1 with free-dim sum into ACC[:, col]
                nc.vector.scalar_tensor_tensor(
                    out=out if out is not None else JUNK,
                    in0=in0, scalar=scale, in1=in1_,
                    op0=OP.mult, op1=OP.mult,
                    accum_out=ACC[:, col:col + 1])

            # ================= trig (needs IN1 only) =================
            HV = pool.tile([P, 4, FW], F16)       # [lht, wht, lhp, whp]
            nc.vector.tensor_scalar(out=HV, in0=_ap(IN1, 5, [(-1, 4)]),
                                    scalar1=0.5, scalar2=None, op0=OP.mult)
            DXY = pool.tile([P, 2, FW], F16)      # [dx, dy]
            nc.vector.tensor_tensor(out=DXY, in0=IN1[:, 6:8, :],
                                    in1=IN1[:, 8:10, :], op=OP.subtract)

            PIB = spool.tile([P, 1], F32)
            nc.vector.memset(PIB, 1.5707963267948966)
            TR = pool.tile([P, 4, FW], F16)       # [cp, sp, ct, st]
            nc.scalar.activation(_ap(TR, 1, [(2, 2)]), IN1[:, 0:2, :], AF.Sin)
            nc.scalar.activation(_ap(TR, 0, [(2, 2)]), IN1[:, 0:2, :], AF.Sin,
                                 bias=PIB[:, 0:1])
            tap("TR", TR, 4)
            ACS = pool.tile([P, 4, FW], F16)      # |cp| |sp| |ct| |st|
            nc.scalar.activation(ACS, TR, AF.Abs)

            tap("HV", HV, 4)
            tap("DXY", DXY, 2)
            # TP=[cp*ct, sp*st], TQ=[sp*ct, cp*st]
            TP = pool.tile([P, 2, FW], F16, tag="T2a")
            TQ = pool.tile([P, 2, FW], F16, tag="T2b")
            nc.vector.tensor_tensor(out=TP, in0=TR[:, 0:2, :], in1=TR[:, 2:4, :],
                                    op=OP.mult)
            nc.vector.tensor_tensor(out=TQ, in0=_ap(TR, 1, [(-1, 2)]),
                                    in1=TR[:, 2:4, :], op=OP.mult)
            # CS8 = [-cd,-sd,-sd,+cd,+cd,-sd,-sd,-cd]; cd=TP0+TP1, sd=TQ0-TQ1
            CS8 = pool.tile([P, 8, FW], F16, tag="S8a")
            nc.vector.tensor_tensor(out=_ap(CS8, 3, [(1, 2)]),
                                    in0=_ap(TP, 0, [(0, 2)]),
                                    in1=_ap(TP, 1, [(0, 2)]), op=OP.add)
            nc.vector.scalar_tensor_tensor(out=_ap(CS8, 0, [(7, 2)]),
                                           in0=_ap(TP, 0, [(0, 2)]), scalar=-1.0,
                                           in1=_ap(TP, 1, [(0, 2)]),
                                           op0=OP.mult, op1=OP.subtract)
            nc.vector.tensor_tensor(out=_ap(CS8, 1, [(4, 2), (1, 2)]),
                                    in0=_ap(TQ, 1, [(0, 2), (0, 2)]),
                                    in1=_ap(TQ, 0, [(0, 2), (0, 2)]),
                                    op=OP.subtract)

            # DC = [dcx, dcy, dcx2, dcy2]
            PT = pool.tile([P, 4, FW], F16, tag="S4a")
            QT = pool.tile([P, 4, FW], F16, tag="S4b")
            nc.vector.tensor_tensor(out=PT, in0=_ap(DXY, 0, [(0, 2), (1, 2)]),
                                    in1=_ap(TR, 2, [(-2, 2), (0, 2)]), op=OP.mult)
            nc.vector.tensor_tensor(out=QT, in0=_ap(DXY, 0, [(0, 2), (1, 2)]),
                                    in1=_ap(TR, 3, [(-2, 2), (0, 2)]), op=OP.mult)
            DC = pool.tile([P, 4, FW], F16)
            nc.vector.tensor_tensor(out=_ap(DC, 0, [(2, 2)]),
                                    in0=_ap(PT, 0, [(2, 2)]),
                                    in1=_ap(QT, 1, [(2, 2)]), op=OP.add)
            nc.vector.tensor_tensor(out=_ap(DC, 1, [(2, 2)]),
                                    in0=_ap(PT, 1, [(2, 2)]),
                                    in1=_ap(QT, 0, [(2, 2)]), op=OP.subtract)

            tap("DC", DC, 4)
            # UVXD = CS8 * [lhp,whp,lhp,whp,lht,wht,lht,wht]  (= D/2)
            UVXD = pool.tile([P, 8, FW], F16, tag="S8b")
            nc.vector.tensor_tensor(out=UVXD, in0=CS8,
                                    in1=_ap(HV, 2, [(-2, 2), (0, 2), (1, 2)]),
                                    op=OP.mult)

            tap("UVXD", UVXD, 8)
            # SC8 = [sA, sC, sB, sD, pA, pB, pC, pD]
            SC8 = pool.tile([P, 8, FW], F16, tag="S8c")
            nc.vector.scalar_tensor_tensor(out=_ap(SC8, 0, [(1, 2)]),
                                           in0=_ap(UVXD, 0, [(2, 2)]), scalar=-1.0,
                                           in1=_ap(UVXD, 1, [(2, 2)]),
                                           op0=OP.mult, op1=OP.subtract)
            nc.vector.tensor_tensor(out=_ap(SC8, 2, [(1, 2)]),
                                    in0=_ap(UVXD, 1, [(2, 2)]),
                                    in1=_ap(UVXD, 0, [(2, 2)]), op=OP.subtract)
            nc.vector.tensor_tensor(out=SC8[:, 4, :], in0=UVXD[:, 4, :],
                                    in1=UVXD[:, 5, :], op=OP.add)
            nc.vector.tensor_tensor(out=SC8[:, 5, :], in0=UVXD[:, 4, :],
                                    in1=UVXD[:, 5, :], op=OP.subtract)
            nc.vector.scalar_tensor_tensor(out=SC8[:, 6, :], in0=UVXD[:, 6, :],
                                           scalar=-1.0, in1=UVXD[:, 7, :],
                                           op0=OP.mult, op1=OP.subtract)
            nc.vector.tensor_tensor(out=SC8[:, 7, :], in0=UVXD[:, 7, :],
                                    in1=UVXD[:, 6, :], op=OP.subtract)

            tap("SC8", SC8, 8)
            # corners: CRN = [AX0..3, AY0..3, BX0..3, BY0..3]
            CRN = pool.tile([P, 16, FW], F16, tag="B16a")

            def corner2(dst0, dstep, dcslot, scslot, scstep, op):
                nc.vector.tensor_tensor(
                    out=_ap(CRN, dst0, [(dstep, 2)]),
                    in0=_ap(DC, dcslot, [(0, 2)]),
                    in1=_ap(SC8, scslot, [(scstep, 2)]), op=op)

            corner2(0, 3, 0, 0, 2, OP.add)        # AX0=dcx+sA, AX3=dcx+sB
            corner2(1, 1, 0, 2, -2, OP.subtract)  # AX1=dcx-sB, AX2=dcx-sA
            corner2(4, 3, 1, 1, 2, OP.add)        # AY0=dcy+sC, AY3=dcy+sD
            corner2(5, 1, 1, 3, -2, OP.subtract)  # AY1=dcy-sD, AY2=dcy-sC
            corner2(8, 3, 2, 4, 1, OP.subtract)   # BX0=dcx2-pA, BX3=dcx2-pB
            corner2(9, 1, 2, 5, -1, OP.add)       # BX1=dcx2+pB, BX2=dcx2+pA
            corner2(12, 3, 3, 6, 1, OP.add)       # BY0=dcy2+pC, BY3=dcy2+pD
            corner2(13, 1, 3, 7, -1, OP.subtract) # BY1=dcy2-pD, BY2=dcy2-pC

            # ============ clip: reciprocals from UVXD ============
            UVX32 = pool.tile([P, 8, FW], F32, tag="F32a")
            nc.scalar.activation(UVX32, UVXD, AF.Copy, bias=1e-12)
            REC32 = UVX32
            nc.vector.reciprocal_approx_fast(
                out=REC32.rearrange("p a b -> p (a b)"),
                in_=UVX32.rearrange("p a b -> p (a b)"))
            # r = REC/2 clamped to +-8000
            nc.vector.tensor_scalar(out=REC32, in0=REC32, scalar1=0.5,
                                    scalar2=-8000.0, op0=OP.mult, op1=OP.max)
            RD8 = pool.tile([P, 8, FW], F16, tag="S8b")
            nc.vector.tensor_scalar(out=RD8, in0=REC32, scalar1=8000.0,
                                    scalar2=None, op0=OP.min)
            tap("RD8", RD8, 8)
            RA = pool.tile([P, 8, FW], F16, tag="S8a")
            nc.scalar.activation(RA, RD8, AF.Abs)
            RL = pool.tile([P, 8, FW], F16, tag="S8c")
            nc.vector.tensor_tensor(out=RL, in0=RA,
                                    in1=_ap(HV, 0, [(1, 4), (0, 2)]), op=OP.mult)

            tap("RL", RL, 8)
            rep16 = [(2, 4), (0, 2), (1, 2)]
            P16 = pool.tile([P, 16, FW], F16, tag="B16b")
            nc.vector.tensor_tensor(out=P16, in0=CRN,
                                    in1=_ap(RD8, 0, rep16), op=OP.mult)
            OPA = pool.tile([P, 16, FW], F16, tag="B16a")   # reuse CRN buffer
            nc.vector.tensor_tensor(out=OPA, in0=_ap(RL, 0, rep16), in1=P16,
                                    op=OP.subtract)
            OPB = P16   # in-place: OPB = RL16rep + P16 overwrites P16
            nc.vector.tensor_tensor(out=OPB, in0=_ap(RL, 0, rep16), in1=P16,
                                    op=OP.add)

            # T1m/T0m: min over the two axes; edges {0,1} vs {2,3} swap A/B roles
            T1m = pool.tile([P, 8, FW], F16, tag="S8a")
            T0m = pool.tile([P, 8, FW], F16, tag="S8b")
            nc.vector.tensor_tensor(out=_ap(T1m, 0, [(4, 2), (1, 2)]),
                                    in0=_ap(OPA, 0, [(8, 2), (1, 2)]),
                                    in1=_ap(OPA, 4, [(8, 2), (1, 2)]), op=OP.min)
            nc.vector.tensor_tensor(out=_ap(T1m, 2, [(4, 2), (1, 2)]),
                                    in0=_ap(OPB, 2, [(8, 2), (1, 2)]),
                                    in1=_ap(OPB, 6, [(8, 2), (1, 2)]), op=OP.min)
            nc.vector.tensor_tensor(out=_ap(T0m, 0, [(4, 2), (1, 2)]),
                                    in0=_ap(OPB, 0, [(8, 2), (1, 2)]),
                                    in1=_ap(OPB, 4, [(8, 2), (1, 2)]), op=OP.min)
            nc.vector.tensor_tensor(out=_ap(T0m, 2, [(4, 2), (1, 2)]),
                                    in0=_ap(OPA, 2, [(8, 2), (1, 2)]),
                                    in1=_ap(OPA, 6, [(8, 2), (1, 2)]), op=OP.min)
            # SEG = relu(min(T1,1) - relu(-T0m))
            nc.scalar.activation(T0m, T0m, AF.Relu, scale=-1.0)
            nc.vector.tensor_scalar(out=T1m, in0=T1m, scalar1=1.0,
                                    scalar2=None, op0=OP.min)
            SEG = pool.tile([P, 8, FW], F16, tag="S8c")
            nc.vector.tensor_tensor(out=SEG, in0=T1m, in1=T0m, op=OP.subtract)
            nc.scalar.activation(SEG, SEG, AF.Relu)

            tap("SEG", SEG, 8)
            # ============ intersection (closed-form cross sum) ============
            SD1 = pool.tile([P, 2, FW], F16, tag="T2a")   # [S0-S2, S1-S3]
            nc.vector.tensor_tensor(out=SD1, in0=_ap(SEG, 0, [(1, 2)]),
                                    in1=_ap(SEG, 2, [(1, 2)]), op=OP.subtract)
            SALL = pool.tile([P, 4, FW], F16, tag="S4a")  # [S0+S2,S1+S3,S4+S6,S5+S7]
            nc.vector.tensor_tensor(out=SALL, in0=_ap(SEG, 0, [(4, 2), (1, 2)]),
                                    in1=_ap(SEG, 2, [(4, 2), (1, 2)]), op=OP.add)
            SS2 = pool.tile([P, 2, FW], F16, tag="T2b")   # [sumS_A, sumS_B]
            nc.vector.tensor_tensor(out=SS2, in0=_ap(SALL, 0, [(2, 2)]),
                                    in1=_ap(SALL, 1, [(2, 2)]), op=OP.add)
            Pm = pool.tile([P, 2, FW], F16, tag="T2c")    # [dcy2*SD0, dcx2*SD1]
            nc.vector.tensor_tensor(out=Pm, in0=SD1,
                                    in1=_ap(DC, 3, [(-1, 2)]), op=OP.mult)
            nc.vector.tensor_tensor(out=Pm, in0=Pm,
                                    in1=_ap(HV, 2, [(1, 2)]), op=OP.mult)
            AREA2 = pool.tile([P, 2, FW], F16, tag="A2")  # [lhp*whp, lht*wht]
            nc.vector.tensor_tensor(out=AREA2, in0=_ap(HV, 2, [(-2, 2)]),
                                    in1=_ap(HV, 3, [(-2, 2)]), op=OP.mult)
            MM2 = pool.tile([P, 2, FW], F16, tag="T2d")
            nc.vector.tensor_tensor(out=MM2, in0=AREA2, in1=SS2, op=OP.mult)
            nc.vector.tensor_tensor(out=Pm, in0=Pm, in1=MM2, op=OP.subtract)
            HACA = pool.tile([P, FW], F16, tag="K1")
            nc.vector.tensor_tensor(out=HACA, in0=Pm[:, 0, :], in1=Pm[:, 1, :],
                                    op=OP.add)
            INTER = pool.tile([P, FW], F16, tag="K2")
            nc.scalar.activation(INTER, HACA, AF.Abs)

            tap("INTER", INTER, 1)
            U1 = pool.tile([P, FW], F16, tag="K3")
            nc.vector.tensor_tensor(out=U1, in0=AREA2[:, 0, :],
                                    in1=AREA2[:, 1, :], op=OP.add)
            UNION = pool.tile([P, FW], F16, tag="K4")
            nc.vector.scalar_tensor_tensor(out=UNION, in0=U1, scalar=4.0,
                                           in1=INTER, op0=OP.mult, op1=OP.subtract)
            ING = INTER
            UC = UNION
            nc.vector.tensor_scalar(out=UC, in0=UNION, scalar1=EPS,
                                    scalar2=None, op0=OP.max)

            # ============ enclosing box + center dist ============
            PA_ = pool.tile([P, 4, FW], F16, tag="S4a")
            PB_ = pool.tile([P, 4, FW], F16, tag="S4b")
            nc.vector.tensor_tensor(out=PA_, in0=_ap(HV, 2, [(-2, 2), (1, 2)]),
                                    in1=ACS, op=OP.mult)
            nc.vector.tensor_tensor(out=PB_, in0=_ap(HV, 2, [(-2, 2), (1, 2)]),
                                    in1=_ap(ACS, 1, [(2, 2), (-1, 2)]), op=OP.mult)
            E2 = pool.tile([P, 4, FW], F16, tag="S4c")    # [exP, exT, eyP, eyT]
            nc.vector.tensor_tensor(out=_ap(E2, 0, [(1, 2)]),
                                    in0=_ap(PA_, 0, [(2, 2)]),
                                    in1=_ap(PA_, 1, [(2, 2)]), op=OP.add)
            nc.vector.tensor_tensor(out=_ap(E2, 2, [(1, 2)]),
                                    in0=_ap(PB_, 0, [(2, 2)]),
                                    in1=_ap(PB_, 1, [(2, 2)]), op=OP.add)
            # Earr = [exP, eyP, exT, eyT]; CEN = [xp, yp, xt, yt]
            XE = pool.tile([P, 4, FW], F16, tag="S4d")
            XD = pool.tile([P, 4, FW], F16, tag="S4e")
            # Earr: slots (0,2,1,3) of E2 => [exP, eyP, exT, eyT]
            Earr = _ap(E2, 0, [(1, 2), (2, 2)])
            nc.vector.tensor_tensor(out=XE, in0=IN1[:, 6:10, :], in1=Earr,
                                    op=OP.add)
            nc.vector.tensor_tensor(out=XD, in0=IN1[:, 6:10, :], in1=Earr,
                                    op=OP.subtract)
            HX = pool.tile([P, 2, FW], F16, tag="T2a")
            LX = pool.tile([P, 2, FW], F16, tag="T2b")
            nc.vector.tensor_tensor(out=HX, in0=_ap(XE, 0, [(1, 2)]),
                                    in1=_ap(XE, 2, [(1, 2)]), op=OP.max)
            nc.vector.tensor_tensor(out=LX, in0=_ap(XD, 0, [(1, 2)]),
                                    in1=_ap(XD, 2, [(1, 2)]), op=OP.min)
            W2 = pool.tile([P, 2, FW], F16, tag="T2c")
            nc.vector.tensor_tensor(out=W2, in0=HX, in1=LX, op=OP.subtract)
            SQ2 = pool.tile([P, 2, FW], F16, tag="T2d")
            nc.scalar.activation(SQ2, W2, AF.Square)
            C2 = pool.tile([P, FW], F16, tag="K8")
            nc.vector.tensor_tensor(out=C2, in0=SQ2[:, 0, :], in1=SQ2[:, 1, :],
                                    op=OP.add)
            nc.vector.tensor_scalar(out=C2, in0=C2, scalar1=EPS,
                                    scalar2=None, op0=OP.max)
            D2P = pool.tile([P, 2, FW], F16, tag="T2e")
            nc.scalar.activation(D2P, DXY, AF.Square)
            D2 = pool.tile([P, FW], F16, tag="K9")
            nc.vector.tensor_tensor(out=D2, in0=D2P[:, 0, :], in1=D2P[:, 1, :],
                                    op=OP.add)

            # DL = (d2*UC - ING*C2) / (C2*UC); one reciprocal
            CM = pool.tile([P, FW], F32, tag="KF1")
            nc.vector.tensor_tensor(out=CM, in0=C2, in1=UC, op=OP.mult)
            RECM = pool.tile([P, FW], F32, tag="KF2")
            nc.vector.reciprocal_approx_fast(out=RECM, in_=CM)
            N1 = D2
            nc.vector.tensor_tensor(out=N1, in0=D2, in1=UC, op=OP.mult)
            N2 = pool.tile([P, FW], F16, tag="K11")
            nc.vector.tensor_tensor(out=N2, in0=ING, in1=C2, op=OP.mult)
            nc.vector.tensor_tensor(out=N1, in0=N1, in1=N2, op=OP.subtract)
            DL = N1
            nc.vector.tensor_tensor(out=DL, in0=N1, in1=RECM, op=OP.mult)
            wm = IN2[:, 11, :]
            acc_sum(DL, wm, 2)

            tap("C2", C2, 1)
            tap("D2", D2, 1)
            tap("DL", DL, 1)
            # ============ smooth L1 (z, h, vx, vy) ============
            DD = pool.tile([P, 4, FW], F16, tag="S4a")
            nc.vector.tensor_tensor(out=DD, in0=_ap(IN2, 0, [(2, 4)]),
                                    in1=_ap(IN2, 1, [(2, 4)]), op=OP.subtract)
            AD = pool.tile([P, 4, FW], F16, tag="S4b")
            nc.scalar.activation(AD, DD, AF.Abs)
            RM = pool.tile([P, 4, FW], F16, tag="S4c")
            nc.scalar.activation(RM, AD, AF.Relu, scale=-1.0, bias=1.0)
            R2h = pool.tile([P, 4, FW], F16, tag="S4d")
            nc.scalar.activation(R2h, RM, AF.Square, scale=0.7071067811865476)
            SL = pool.tile([P, 4, FW], F16, tag="S4e")
            nc.vector.tensor_tensor(out=SL, in0=AD, in1=R2h, op=OP.add)
            for k in range(4):
                acc_sum(SL[:, k, :], wm, 3 + k)

            # ============ BCE on iou head ============
            iop = IN2[:, 8, :]
            BA = pool.tile([P, FW], F16, tag="K13")
            nc.scalar.activation(BA, iop, AF.Abs)
            nc.scalar.activation(BA, BA, AF.Exp, scale=-1.0)
            nc.scalar.activation(BA, BA, AF.Ln, bias=1.0)
            BR = pool.tile([P, FW], F16, tag="K14")
            nc.scalar.activation(BR, iop, AF.Relu)
            BXY = pool.tile([P, FW], F16, tag="K15")
            nc.vector.tensor_tensor(out=BXY, in0=iop, in1=IN2[:, 9, :],
                                    op=OP.mult)
            nc.vector.tensor_tensor(out=BR, in0=BR, in1=BXY, op=OP.subtract)
            nc.vector.tensor_tensor(out=BR, in0=BR, in1=BA, op=OP.add)
            acc_sum(BR, wm, 7)
            nc.scalar.activation(JUNK, wm, AF.Copy, accum_out=ACC[:, 8:9])

            # ============ focal ============
            clsf = IN2[:, 10, :]
            ET = pool.tile([P, 10, FW], F16, tag="S10a")
            nc.scalar.activation(ET, IN3, AF.Exp)
            S5 = pool.tile([P, 5, FW], F16, tag="S5a")
            nc.gpsimd.tensor_tensor(out=S5, in0=ET[:, 0:5, :], in1=ET[:, 5:10, :],
                                    op=OP.add)
            S2 = pool.tile([P, 2, FW], F16, tag="T2a")
            nc.vector.tensor_tensor(out=S2, in0=S5[:, 0:2, :], in1=S5[:, 2:4, :],
                                    op=OP.add)
            SSs = pool.tile([P, FW], F16, tag="K16")
            nc.vector.tensor_tensor(out=SSs, in0=S2[:, 0, :], in1=S2[:, 1, :],
                                    op=OP.add)
            nc.vector.tensor_tensor(out=SSs, in0=SSs, in1=S5[:, 4, :], op=OP.add)
            # one-hot on Vector (cheap 4x-mode TS); gather mult on Pool
            EQ10 = pool.tile([P, 10, FW], F16, tag="S10b")
            for c in range(10):
                nc.vector.tensor_scalar(out=EQ10[:, c, :], in0=clsf,
                                        scalar1=float(c), scalar2=None,
                                        op0=OP.is_equal)
            MT = pool.tile([P, 10, FW], F16, tag="S10a")   # reuse ET buffer
            nc.gpsimd.tensor_tensor(out=MT, in0=EQ10, in1=IN3, op=OP.mult)
            L5 = pool.tile([P, 5, FW], F16, tag="S5a")
            nc.gpsimd.tensor_tensor(out=L5, in0=MT[:, 0:5, :], in1=MT[:, 5:10, :],
                                    op=OP.add)
            L2 = pool.tile([P, 2, FW], F16, tag="T2b")
            nc.vector.tensor_tensor(out=L2, in0=L5[:, 0:2, :], in1=L5[:, 2:4, :],
                                    op=OP.add)
            LT = pool.tile([P, FW], F16, tag="K17")
            nc.vector.tensor_tensor(out=LT, in0=L2[:, 0, :], in1=L2[:, 1, :],
                                    op=OP.add)
            nc.vector.tensor_tensor(out=LT, in0=LT, in1=L5[:, 4, :], op=OP.add)
            LNS = SSs
            nc.scalar.activation(LNS, SSs, AF.Ln)
            LPT = LT
            nc.vector.tensor_tensor(out=LPT, in0=LT, in1=LNS, op=OP.subtract)
            PTT = pool.tile([P, FW], F16, tag="K20")
            nc.scalar.activation(PTT, LPT, AF.Exp)
            OM2 = PTT
            nc.scalar.activation(OM2, PTT, AF.Square, scale=-1.0, bias=1.0)
            F1 = OM2
            nc.vector.tensor_tensor(out=F1, in0=OM2, in1=LPT, op=OP.mult)
            MPOS = pool.tile([P, FW], F16, tag="K23")
            nc.vector.tensor_scalar(out=MPOS, in0=clsf, scalar1=0.5,
                                    scalar2=None, op0=OP.is_gt)
            nc.vector.tensor_scalar(out=MPOS, in0=MPOS, scalar1=-0.5,
                                    scalar2=0.75, op0=OP.mult, op1=OP.add)
            nc.vector.tensor_tensor(out=F1, in0=F1, in1=MPOS, op=OP.mult)
            VLD = pool.tile([P, FW], F16, tag="K24")
            nc.vector.tensor_scalar(out=VLD, in0=clsf, scalar1=-0.5,
                                    scalar2=None, op0=OP.is_ge)
            acc_sum(F1, VLD, 0, scale=-1.0)
            nc.scalar.activation(JUNK, VLD, AF.Copy, accum_out=ACC[:, 1:2])

            # ============ cross-partition reduce + output ============
            PS = ppool.tile([1, 16], F32)
            nc.tensor.matmul(PS, ones, ACC, start=True, stop=True)
            OUT = spool.tile([1, 16], F32)
            nc.scalar.copy(out=OUT, in_=PS)
            nc.sync.dma_start(out=outp[:, :], in_=OUT)
    nc.compile()
    nc._dbg_slots = dbg_slots
    return nc


_NC_CACHE = None


def _get_nc():
    global _NC_CACHE
    if _NC_CACHE is None:
        _NC_CACHE = build_bass()
    return _NC_CACHE


def pack_inputs(cls_pred, reg_pred, iou_pred, reg_targets, iou_targets,
                cls_targets, reg_weights):
    """Returns list of 8 per-core input dicts (in1/in2/in3 fp16 arrays)."""
    B = cls_pred.shape[0]
    maps = []
    for b in range(B):
        rp = np.asarray(reg_pred[b], np.float32).reshape(9, P, FW)
        rt = np.asarray(reg_targets[b], np.float32).reshape(9, P, FW)
        h1 = np.empty((10, P, FW), np.float16)
        h1[0] = rp[6]; h1[1] = rt[6]
        h1[2] = rp[3]; h1[3] = rp[4]
        h1[4] = rt[3]; h1[5] = rt[4]
        h1[6] = rp[0]; h1[7] = rp[1]
        h1[8] = rt[0]; h1[9] = rt[1]
        h2 = np.empty((12, P, FW), np.float16)
        h2[0] = rp[2]; h2[1] = rt[2]
        h2[2] = rp[5]; h2[3] = rt[5]
        h2[4] = rp[7]; h2[5] = rt[7]
        h2[6] = rp[8]; h2[7] = rt[8]
        h2[8] = np.asarray(iou_pred[b], np.float32).reshape(P, FW)
        h2[9] = np.asarray(iou_targets[b], np.float32).reshape(P, FW)
        h2[10] = np.asarray(cls_targets[b]).astype(np.float32).reshape(P, FW)
        h2[11] = np.asarray(reg_weights[b]).astype(np.float32).reshape(P, FW)
        h3 = np.asarray(cls_pred[b], np.float32).reshape(10, P, FW).astype(np.float16)
        maps.append({
            "in1": np.ascontiguousarray(h1.transpose(1, 0, 2)),
            "in2": np.ascontiguousarray(h2.transpose(1, 0, 2)),
            "in3": np.ascontiguousarray(h3.transpose(1, 0, 2)),
        })
    return maps


def combine(parts):
    """parts: [8, 1, 16] per-core raw sums -> final [7] float32."""
    p = np.asarray(parts, np.float64).sum(0).reshape(-1)
    focal_s, valid_s, diou_s, z_s, h_s, vx_s, vy_s, bce_s, w_s = p[:9]
    num_pos = max(w_s, 1.0)
    cls_loss = focal_s / max(valid_s, 1.0)
    bev_loss = (diou_s + w_s) / num_pos
    z_loss = (z_s - 0.5 * w_s) / num_pos
    h_loss = (h_s - 0.5 * w_s) / num_pos
    vel_loss = (vx_s + vy_s - w_s) / num_pos
    iou_loss = bce_s / num_pos
    total = cls_loss + 2.0 * bev_loss + z_loss + h_loss + vel_loss + iou_loss
    return np.array([total, cls_loss, bev_loss, z_loss, h_loss, vel_loss, iou_loss],
                    np.float32)


def kernel(cls_pred, reg_pred, iou_pred, reg_targets, iou_targets,
           cls_targets, reg_weights, _trace=False):
    cls_pred, reg_pred, iou_pred, reg_targets, iou_targets, cls_targets, reg_weights = (
        np.asarray(a) for a in (cls_pred, reg_pred, iou_pred, reg_targets,
                                iou_targets, cls_targets, reg_weights))
    nc = _get_nc()
    in_maps = pack_inputs(cls_pred, reg_pred, iou_pred, reg_targets,
                          iou_targets, cls_targets, reg_weights)
    res = run_bass_kernel_spmd(nc, in_maps, core_ids=list(range(8)), trace=_trace)
    parts = [res.results[i]["out"] for i in range(8)]
    out = combine(parts)
    if _trace:
        return out, res
    return out
